# revision 1
# baseline (speedup 1.0000x reference)
"""GNN message-passing (graph convolution) kernel for 8 Trainium2 NeuronCores.

    out = relu(segment_sum(h[col], row) + bias),  h = x @ W

Strategy (dst-block sharding — no collectives needed):
  * Host sorts edges by destination node and buckets them into 157 blocks of
    128 dst nodes; blocks are assigned contiguously to cores (20/core).  Each
    core produces a disjoint slice of the output, so partial aggregates never
    need an all-reduce.
  * Phase A (per core, replicated): h = x @ W on the PE in fp16
    (PSUM fp32 accumulate), streamed to a per-core DRAM buffer h[20096,128]
    fp16.  x is shipped pre-transposed/pre-tiled from the host so each lhsT
    tile is one contiguous 64KB DMA.
  * Phase B: for each dst block, dma_gather (SWDGE) fetches the h rows of the
    block's (padded) edge list into SBUF with edge-on-partition layout
    [128e, PB, 128f]; the DVE builds one-hot tiles S[e,n] = (iota == rowloc)
    in fp16; the PE computes out_block += S^T @ val accumulating all chunks of
    the block in PSUM fp32 — an exact segment-sum.  Bias is folded in as an
    extra "bias chunk" per block (gathers a bias row stored at h[20095] with an
    identity one-hot).  ACT applies ReLU PSUM->SBUF, then the result is DMA'd
    out.

Numerics: fp16 operands with fp32 accumulation everywhere; one-hot matmul is
exact, so the only error is fp16 rounding of x, W and h (~1e-3 relative).
"""

import sys

import numpy as np

sys.path.insert(0, "/opt/trn_rl_repo")

import concourse.bacc as bacc  # noqa: E402
import concourse.bass as bass  # noqa: E402  (engine types)
import concourse.mybir as mybir  # noqa: E402
from concourse.bass_utils import run_bass_kernel_spmd  # noqa: E402

N_NODES = 20000
FIN = 256
FOUT = 128
N_EDGES = 640000

NT = 157                 # node tiles of 128 (nodes padded to 20096)
NPAD = NT * 128          # 20096
NBLK = 157               # dst blocks of 128 nodes
NCORES = 8
NB = 20                  # block slots per core (core 7: 17 real + 3 dummy)
BIAS_ROW = NPAD - 1      # h row that phase-B reads the bias vector from

XT_BUFS = 4              # xT tile ring (phase A)
H_BUFS = 4               # h sbuf tile ring (phase A)
S_BUFS = 4               # one-hot tile ring (phase B)

FP16 = mybir.dt.float16
FP32 = mybir.dt.float32
I16 = mybir.dt.int16


def _host_prep(x, edge_index, weight, bias):
    """Cast/retile operands and bucket edges by destination block."""
    x = np.asarray(x, np.float32)
    weight = np.asarray(weight, np.float32)
    bias = np.asarray(bias, np.float32)

    xpad = np.zeros((NPAD, FIN), np.float32)
    xpad[:N_NODES] = x
    # lhsT tiles: xt_tiles[i, k, kc, n] = x[i*128+n, kc*128+k]
    xt_tiles = np.ascontiguousarray(
        xpad.reshape(NT, 128, 2, 128).transpose(0, 3, 2, 1).astype(np.float16)
    )
    w_t = np.ascontiguousarray(weight.astype(np.float16).reshape(2, 128, 128))
    bias16 = np.ascontiguousarray(bias.astype(np.float16).reshape(1, 128))
    iota16 = np.ascontiguousarray(
        np.broadcast_to(np.arange(128, dtype=np.float16), (128, 128))
    )

    row = np.asarray(edge_index[0]).astype(np.int64)
    col = np.asarray(edge_index[1]).astype(np.int64)
    order = np.argsort(row, kind="stable")
    rs = row[order].astype(np.int32)
    cs = col[order].astype(np.int32)

    blk = rs >> 7
    counts = np.bincount(blk, minlength=NBLK)
    starts = np.concatenate([[0], np.cumsum(counts)])
    pb = int(np.max((counts + 127) // 128)) + 1  # +1 for the bias chunk
    pb = ((pb + 6) // 7) * 7  # sub-gathers of 7 chunks (896 idxs <= SWDGE ring)
    nidx = pb * 128
    idxc = nidx // 16

    col16 = np.zeros((NCORES, 128, NB * idxc), np.int16)
    rloc16 = np.full((NCORES, 128, NB * pb), -1.0, np.float32)
    bias_rl = np.arange(128, dtype=np.float32)
    for c in range(NCORES):
        for s in range(NB):
            b = c * NB + s
            lin_col = np.zeros(nidx, np.int32)
            lin_rl = np.full(nidx, -1.0, np.float32)
            lin_col[:128] = BIAS_ROW          # bias chunk: identity one-hot
            lin_rl[:128] = bias_rl
            if b < NBLK:
                e0, e1 = int(starts[b]), int(starts[b + 1])
                k = e1 - e0
                lin_col[128:128 + k] = cs[e0:e1]
                lin_rl[128:128 + k] = rs[e0:e1] - b * 128
            # the SWDGE tx/rx Q7 pair read the indices from different
            # 16-partition groups — replicate the 16-row wrap to all 128
            col16[c, :, s * idxc:(s + 1) * idxc] = np.tile(
                lin_col.reshape(idxc, 16).T.astype(np.int16), (8, 1)
            )
            rloc16[c, :, s * pb:(s + 1) * pb] = (
                lin_rl.reshape(pb, 128).T.astype(np.float32)
            )
    return xt_tiles, w_t, bias16, iota16, col16, rloc16, pb


def _build_program(pb):
    nidx = pb * 128
    idxc = nidx // 16
    nc = bacc.Bacc("TRN2")

    xt_d = nc.dram_tensor("xt", [NT, 128, 2, 128], FP16, kind="ExternalInput")
    w_d = nc.dram_tensor("w", [2, 128, 128], FP16, kind="ExternalInput")
    b_d = nc.dram_tensor("bias", [1, 128], FP16, kind="ExternalInput")
    io_d = nc.dram_tensor("iota", [128, 128], FP16, kind="ExternalInput")
    col_d = nc.dram_tensor("col", [128, NB * idxc], I16, kind="ExternalInput")
    rl_d = nc.dram_tensor("rl", [128, NB * pb], FP32, kind="ExternalInput")
    h_d = nc.dram_tensor("hbuf", [NPAD, 128], FP16)
    o_d = nc.dram_tensor("out", [NB * 128, 128], FP32, kind="ExternalOutput")

    from contextlib import ExitStack

    with ExitStack() as es:
        ph0 = es.enter_context(nc.psum_tensor("ph0", [128, 512], FP32))
        ph1 = es.enter_context(nc.psum_tensor("ph1", [128, 512], FP32))
        ph2 = es.enter_context(nc.psum_tensor("ph2", [128, 512], FP32))
        ph3 = es.enter_context(nc.psum_tensor("ph3", [128, 512], FP32))
        pb0 = es.enter_context(nc.psum_tensor("pb0", [128, 512], FP32))
        pb1 = es.enter_context(nc.psum_tensor("pb1", [128, 512], FP32))
        w_sb = es.enter_context(nc.sbuf_tensor("w_sb", [128, 2, 128], FP16))
        xt_sb = es.enter_context(
            nc.sbuf_tensor("xt_sb", [128, XT_BUFS, 2, 128], FP16)
        )
        h_sb = es.enter_context(nc.sbuf_tensor("h_sb", [128, H_BUFS, 128], FP16))
        iota_sb = es.enter_context(nc.sbuf_tensor("iota_sb", [128, 128], FP16))
        col_sb = es.enter_context(nc.sbuf_tensor("col_sb", [128, NB * idxc], I16))
        rl_sb = es.enter_context(nc.sbuf_tensor("rl_sb", [128, NB * pb], FP32))
        val_sb = es.enter_context(
            nc.sbuf_tensor("val_sb", [128, 2, pb, 128], FP16)
        )
        s_sb = es.enter_context(nc.sbuf_tensor("s_sb", [128, S_BUFS, 128], FP16))
        o_sb = es.enter_context(nc.sbuf_tensor("o_sb", [128, 2, 128], FP32))
        # DMA-completion sems rotate per ring slot (DMA completions on one
        # sem can reorder, so each slot gets its own counter).
        s_ld = [es.enter_context(nc.semaphore(f"s_ld{k}")) for k in range(5)]
        s_xt = [es.enter_context(nc.semaphore(f"s_xt{k}")) for k in range(XT_BUFS)]
        s_hw = [es.enter_context(nc.semaphore(f"s_hw{k}")) for k in range(H_BUFS)]
        s_bw = es.enter_context(nc.semaphore("s_bw"))
        s_gat = [
            es.enter_context(nc.semaphore(f"s_gat{k}"))
            for k in range(2 * (pb // 7))
        ]
        s_ow = [es.enter_context(nc.semaphore(f"s_ow{k}")) for k in range(2)]
        # compute-engine sems increment in program order (no ambiguity)
        s_hmm = es.enter_context(nc.semaphore("s_hmm"))
        s_hcp = es.enter_context(nc.semaphore("s_hcp"))
        s_s = es.enter_context(nc.semaphore("s_s"))
        s_pmm = es.enter_context(nc.semaphore("s_pmm"))
        s_ocp = es.enter_context(nc.semaphore("s_ocp"))
        block = es.enter_context(nc.Block())
        ph = [ph0, ph1, ph2, ph3]
        pbk = [pb0, pb1]

        hw_total = [16 * len(range(k, NT, H_BUFS)) for k in range(H_BUFS)]

        def store_h(sync, j):
            sync.wait_ge(s_hcp, j + 1)
            sync.dma_start(
                h_d[j * 128:(j + 1) * 128, :], h_sb[:, j % H_BUFS, :]
            ).then_inc(s_hw[j % H_BUFS], 16)

        @block.sync
        def _(sync):
            # one-time loads
            sync.dma_start(w_sb[:, 0, :], w_d[0]).then_inc(s_ld[0], 16)
            sync.dma_start(w_sb[:, 1, :], w_d[1]).then_inc(s_ld[1], 16)
            sync.dma_start(iota_sb[:, :], io_d[:, :]).then_inc(s_ld[2], 16)
            sync.dma_start(col_sb[:, :], col_d[:, :]).then_inc(s_ld[3], 16)
            sync.dma_start(rl_sb[:, :], rl_d[:, :]).then_inc(s_ld[4], 16)
            # phase A: stream xT tiles in, h tiles out (staggered)
            for i in range(NT):
                if i >= XT_BUFS:
                    sync.wait_ge(s_hmm, i - (XT_BUFS - 1))
                sync.dma_start(xt_sb[:, i % XT_BUFS, :, :], xt_d[i]).then_inc(
                    s_xt[i % XT_BUFS], 16
                )
                if i >= 3:
                    store_h(sync, i - 3)
            for j in range(NT - 3, NT):
                store_h(sync, j)
            # bias row (after ALL h writes are complete — tile 156 covers it)
            for k in range(H_BUFS):
                sync.wait_ge(s_hw[k], hw_total[k])
            sync.dma_start(h_d[BIAS_ROW:BIAS_ROW + 1, :], b_d[0:1, :]).then_inc(
                s_bw, 16
            )
            # phase B: output stores
            for b in range(NB):
                sync.wait_ge(s_ocp, b + 1)
                sync.dma_start(
                    o_d[b * 128:(b + 1) * 128, :], o_sb[:, b % 2, :]
                ).then_inc(s_ow[b % 2], 16)

        @block.gpsimd
        def _(gpsimd):
            gpsimd.wait_ge(s_ld[3], 16)
            for k in range(H_BUFS):
                gpsimd.wait_ge(s_hw[k], hw_total[k])
            gpsimd.wait_ge(s_bw, 16)
            for b in range(NB):
                if b >= 2:
                    gpsimd.wait_ge(s_pmm, (b - 1) * pb)
                for g in range(pb // 7):
                    gpsimd.dma_gather(
                        val_sb[:, b % 2, g * 7:(g + 1) * 7, :],
                        h_d[:, :],
                        col_sb[:, b * idxc + g * 56:b * idxc + (g + 1) * 56],
                        896,
                        896,
                        128,
                    ).then_inc(s_gat[(b % 2) * (pb // 7) + g], 16)

        @block.tensor
        def _(tensor):
            for k in range(2):
                tensor.wait_ge(s_ld[k], 16)
            # phase A: h tile i = xT_i^T @ W  (two K chunks)
            for i in range(NT):
                tensor.wait_ge(s_xt[i % XT_BUFS], 16 * (i // XT_BUFS + 1))
                if i >= XT_BUFS:
                    tensor.wait_ge(s_hcp, i - (XT_BUFS - 1))
                tensor.matmul(
                    ph[i % XT_BUFS][:, 0:128],
                    xt_sb[:, i % XT_BUFS, 0, :],
                    w_sb[:, 0, :],
                    start=True,
                    stop=False,
                )
                tensor.matmul(
                    ph[i % XT_BUFS][:, 0:128],
                    xt_sb[:, i % XT_BUFS, 1, :],
                    w_sb[:, 1, :],
                    start=False,
                    stop=True,
                ).then_inc(s_hmm, 1)
            # phase B: out_block += S_chunk^T @ val_chunk
            for b in range(NB):
                if b >= 2:
                    tensor.wait_ge(s_ocp, b - 1)
                for c in range(pb):
                    j = b * pb + c
                    if c % 7 == 0:
                        tensor.wait_ge(
                            s_gat[(b % 2) * (pb // 7) + c // 7],
                            16 * (b // 2 + 1),
                        )
                    tensor.wait_ge(s_s, j + 1)
                    tensor.matmul(
                        pbk[b % 2][:, 0:128],
                        s_sb[:, j % S_BUFS, :],
                        val_sb[:, b % 2, c, :],
                        start=(c == 0),
                        stop=(c == pb - 1),
                    ).then_inc(s_pmm, 1)

        @block.vector
        def _(vector):
            # phase A: PSUM fp32 -> SBUF fp16
            for i in range(NT):
                vector.wait_ge(s_hmm, i + 1)
                if i >= H_BUFS:
                    vector.wait_ge(s_hw[i % H_BUFS], 16 * (i // H_BUFS))
                vector.tensor_copy(
                    h_sb[:, i % H_BUFS, :], ph[i % XT_BUFS][:, 0:128]
                ).then_inc(s_hcp, 1)
            # phase B: one-hot tiles S[e, n] = (iota[n] == rowloc[e])
            vector.wait_ge(s_ld[2], 16)
            vector.wait_ge(s_ld[4], 16)
            for j in range(NB * pb):
                if j >= S_BUFS:
                    vector.wait_ge(s_pmm, j - (S_BUFS - 1))
                vector.tensor_scalar(
                    s_sb[:, j % S_BUFS, :],
                    iota_sb[:, :],
                    rl_sb[:, j:j + 1],
                    None,
                    mybir.AluOpType.is_equal,
                ).then_inc(s_s, 1)

        @block.scalar
        def _(scalar):
            for b in range(NB):
                scalar.wait_ge(s_pmm, (b + 1) * pb)
                if b >= 2:
                    scalar.wait_ge(s_ow[b % 2], 16 * (b // 2))
                scalar.activation(
                    o_sb[:, b % 2, :],
                    pbk[b % 2][:, 0:128],
                    mybir.ActivationFunctionType.Relu,
                ).then_inc(s_ocp, 1)

    nc.compile()
    return nc


def _run(x, edge_index, weight, bias, trace=False):
    xt_tiles, w_t, bias16, iota16, col16, rloc16, pb = _host_prep(
        x, edge_index, weight, bias
    )
    nc = _build_program(pb)
    in_maps = [
        {
            "xt": xt_tiles,
            "w": w_t,
            "bias": bias16,
            "iota": iota16,
            "col": np.ascontiguousarray(col16[c]),
            "rl": np.ascontiguousarray(rloc16[c]),
        }
        for c in range(NCORES)
    ]
    res = run_bass_kernel_spmd(nc, in_maps, list(range(NCORES)), trace=trace)
    out = np.concatenate([res.results[c]["out"] for c in range(NCORES)], axis=0)
    return np.ascontiguousarray(out[:N_NODES]), res


def kernel(x, edge_index, weight, bias):
    out, _ = _run(x, edge_index, weight, bias, trace=False)
    return out



# revision 22
# speedup vs baseline: 1.8113x; 1.8113x over previous
"""GNN message-passing (graph convolution) kernel for 8 Trainium2 NeuronCores.

    out = relu(segment_sum(h[col], row) + bias),  h = x @ W

Strategy (dst-block sharding -- no collectives needed):
  * Host sorts edges by destination node and buckets them into 157 blocks of
    128 dst nodes.  Blocks are sorted by edge count and dealt snake-wise into
    20 slots x 8 cores so that slot s holds 8 similarly-sized blocks; the
    per-slot chunk count pb_s = max ceil(cnt/128) over its blocks is a program
    constant shared by all cores (SPMD), minimizing padding.
  * Phase A (per core, replicated): h = x @ W on the PE in fp16 (PSUM fp32
    accumulate).  x is shipped pre-transposed [kk, tile, k, n] so the whole
    10.5MB loads in 16 large DMAs; h stays IN SBUF [128, 160*128] fp16 --
    never written to DRAM.
  * Phase B: SWDGE dma_gather with SBUF source (transpose=True, tokens=128)
    fetches 2048 edge rows per gather into valT [128f, 2048e]; the PE
    transposes each 128-edge chunk back to edge-major via an identity matmul
    (4 chunks share one PSUM bank), DVE/ACT copy the bank to SBUF, the DVE
    builds one-hot tiles S[e,n] = (iota == rowloc), and the PE accumulates
    out_slot += S^T @ val over all chunks of the slot in PSUM fp32 -- an
    exact segment-sum.  The bias is folded in as one extra matmul per slot
    with constant operands (identity x bias-broadcast): no gather, no DVE.
  * ACT applies ReLU PSUM->SBUF fp16; output stores are batched 2 blocks per
    DMA.  The host scatters block rows back to their original positions.

Numerics: fp16 operands with fp32 accumulation everywhere; the one-hot and
transpose matmuls are exact, so the only error is fp16 rounding of x, W, h
and the output (~1e-3 relative).
"""

import sys

import numpy as np

sys.path.insert(0, "/opt/trn_rl_repo")

import concourse.bacc as bacc  # noqa: E402
import concourse.mybir as mybir  # noqa: E402
from concourse.bass_utils import run_bass_kernel_spmd  # noqa: E402

N_NODES = 20000
FIN = 256
FOUT = 128
N_EDGES = 640000

NT = 157                 # real node tiles of 128
NTP = 160                # padded tiles (x zero-padded) -> h_sb rows 20480
NPAD = NTP * 128
NBLK = 157               # dst blocks of 128 nodes
NCORES = 8
NSLOT = 20               # block slots per core (slot 19: 5 real + 3 dummy)
NIDX = 1024              # idxs per dma_gather (8 chunks)
CPG = NIDX // 128        # chunks per gather
SCRATCH = 16384          # stock SWDGE ring (1024 descriptors)

FP16 = mybir.dt.float16
FP32 = mybir.dt.float32
I16 = mybir.dt.int16


def _plan(edge_index):
    """Sort/bucket edges; derive the SPMD-uniform slot structure."""
    row = np.asarray(edge_index[0]).astype(np.int64)
    col = np.asarray(edge_index[1]).astype(np.int64)
    order = np.argsort(row, kind="stable")
    rs = row[order].astype(np.int32)
    cs = col[order].astype(np.int32)

    blk = rs >> 7
    counts = np.bincount(blk, minlength=NBLK)
    starts = np.concatenate([[0], np.cumsum(counts)])

    big_first = np.argsort(counts, kind="stable")[::-1]  # block ids by size desc
    pbs = []
    slot_block = np.full((NCORES, NSLOT), -1, np.int64)
    for s in range(NSLOT):
        grp = big_first[s * NCORES:(s + 1) * NCORES]
        pbs.append(int(((counts[grp] + 127) // 128).max()))
        for c, b in enumerate(grp):
            slot_block[c, s] = b
    cum = np.concatenate([[0], np.cumsum(pbs)])
    nch = int(cum[-1])
    cpg = NIDX // 128
    ng = (nch + cpg - 1) // cpg
    nchp = ng * cpg
    return rs, cs, starts, slot_block, pbs, cum, nch, nchp, ng


def _host_prep(x, edge_index, weight, bias):
    """Cast/retile operands; build per-core gather index / rowloc tables."""
    x = np.asarray(x, np.float32)
    weight = np.asarray(weight, np.float32)
    bias = np.asarray(bias, np.float32)

    rs, cs, starts, slot_block, pbs, cum, nch, nchp, ng = _plan(edge_index)

    xpad = np.zeros((NPAD, FIN), np.float32)
    xpad[:N_NODES] = x
    # xt[kk, tile*256 + k*128 + n] = x[tile*128 + n, k*128 + kk]
    xt = np.ascontiguousarray(
        xpad.reshape(NTP, 128, 2, 128)        # [tile, n, k, kk]
        .transpose(3, 0, 2, 1)                 # [kk, tile, k, n]
        .reshape(128, NTP * 2 * 128)
        .astype(np.float16)
    )
    # cst[:, 0:2, :] = W chunks; 2: iota; 3: identity; 4: bias broadcast
    cst = np.zeros((128, 5, 128), np.float16)
    cst[:, 0:2, :] = weight.reshape(2, 128, 128).transpose(1, 0, 2)
    cst[:, 2, :] = np.arange(128, dtype=np.float16)[None, :]
    cst[:, 3, :] = np.eye(128, dtype=np.float16)
    cst[:, 4, :] = bias.astype(np.float16)[None, :]

    gmax = np.zeros(ng, np.int64)
    col16 = np.zeros((NCORES, 128, ng * (NIDX // 16)), np.int16)
    rl32 = np.full((NCORES, 128, nchp), -1.0, np.float32)
    for c in range(NCORES):
        lin_col = np.zeros(nchp * 128, np.int32)
        lin_rl = np.full(nchp * 128, -1.0, np.float32)
        for s in range(NSLOT):
            b = slot_block[c, s]
            if b < 0:
                continue
            e0, e1 = int(starts[b]), int(starts[b + 1])
            k = e1 - e0
            j0 = int(cum[s]) * 128
            o = np.argsort(cs[e0:e1], kind="stable")
            lin_col[j0:j0 + k] = cs[e0:e1][o]
            lin_rl[j0:j0 + k] = (rs[e0:e1] - b * 128)[o]
        # SWDGE idx layout: idx i -> partition i%16, column i//16 (x8 repl.)
        col16[c] = np.tile(
            lin_col.reshape(nchp * 128 // 16, 16).T.astype(np.int16), (8, 1)
        )
        rl32[c] = lin_rl.reshape(nchp, 128).T
        gmax = np.maximum(gmax, lin_col.reshape(ng, NIDX).max(axis=1))

    # per-gather h-frontier gate: h DRAM stores of 16 tiles (2048 rows)
    gates = [int(v) for v in (gmax // 2048 + 1)]
    meta = dict(
        pbs=pbs, cum=[int(v) for v in cum], nch=nch, nchp=nchp, ng=ng,
        gates=gates,
    )
    common = {"xt": xt, "cst": cst}
    per_core = [
        {"col": np.ascontiguousarray(col16[c]), "rl": np.ascontiguousarray(rl32[c])}
        for c in range(NCORES)
    ]
    return common, per_core, slot_block, meta


def _build_program(meta):
    pbs, cum = meta["pbs"], meta["cum"]
    nch, nchp, ng = meta["nch"], meta["nchp"], meta["ng"]
    gates = meta["gates"]
    chunk_slot = []                  # chunk j -> (slot, c)
    for s in range(NSLOT):
        for c in range(pbs[s]):
            chunk_slot.append((s, c))

    # cumulative segsum+bias matmul count after chunk j
    mm_after = []
    tot = 0
    for j in range(nch):
        s, c = chunk_slot[j]
        tot += 1
        if c == pbs[s] - 1:
            tot += 1
        mm_after.append(tot)

    NST = NTP * 128 // 2048          # h DRAM stores (16 tiles each)
    # emit h store k right after the x load that covers its tiles
    store_after_load = {}
    for k in range(NST):
        L = -(-(16 * (k + 1)) // 10) - 1
        store_after_load.setdefault(L, []).append(k)

    nc = bacc.Bacc("TRN2", dynamic_dma_scratch_size=SCRATCH)

    xt_d = nc.dram_tensor("xt", [128, NTP * 2 * 128], FP16, kind="ExternalInput")
    cst_d = nc.dram_tensor("cst", [128, 5, 128], FP16, kind="ExternalInput")
    col_d = nc.dram_tensor("col", [128, ng * (NIDX // 16)], I16, kind="ExternalInput")
    rl_d = nc.dram_tensor("rl", [128, nchp], FP32, kind="ExternalInput")
    h_d = nc.dram_tensor("hbuf", [NTP * 128, 128], FP16)
    o_d = nc.dram_tensor("out", [NSLOT * 128, 128], FP16, kind="ExternalOutput")

    from contextlib import ExitStack

    with ExitStack() as es:
        pha = [es.enter_context(nc.psum_tensor(f"pha{k}", [128, 512], FP32)) for k in range(4)]
        po = [es.enter_context(nc.psum_tensor(f"po{k}", [128, 512], FP32)) for k in range(4)]
        xt_sb = es.enter_context(nc.sbuf_tensor("xt_sb", [128, 4, 10, 2, 128], FP16))
        cst_sb = es.enter_context(nc.sbuf_tensor("cst_sb", [128, 5, 128], FP16))
        h_sb = es.enter_context(nc.sbuf_tensor("h_sb", [128, NTP * 128], FP16))
        val_eb = es.enter_context(nc.sbuf_tensor("val_eb", [128, 4, CPG, 128], FP16))
        s_sb = es.enter_context(nc.sbuf_tensor("s_sb", [128, 16, 128], FP16))
        o_sb = es.enter_context(nc.sbuf_tensor("o_sb", [128, 2, 128], FP16))
        col_sb = es.enter_context(nc.sbuf_tensor("col_sb", [128, ng * (NIDX // 16)], I16))
        rl_sb = es.enter_context(nc.sbuf_tensor("rl_sb", [128, nchp], FP32))

        s_x = [es.enter_context(nc.semaphore(f"s_x{k}")) for k in range(4)]
        s_ld = es.enter_context(nc.semaphore("s_ld"))
        s_hmm = es.enter_context(nc.semaphore("s_hmm"))
        s_hcp = es.enter_context(nc.semaphore("s_hcp"))
        s_hst = [es.enter_context(nc.semaphore(f"s_hst{k}")) for k in range(2)]
        s_gat = [es.enter_context(nc.semaphore(f"s_gat{k}")) for k in range(4)]
        s_s = es.enter_context(nc.semaphore("s_s"))
        s_smm = es.enter_context(nc.semaphore("s_smm"))
        s_act = es.enter_context(nc.semaphore("s_act"))
        s_ost = [es.enter_context(nc.semaphore(f"s_ost{k}")) for k in range(2)]
        block = es.enter_context(nc.Block())

        @block.sync
        def _(sync):
            sync.dma_start(cst_sb[:, :, :], cst_d[:, :, :]).then_inc(s_ld, 16)
            sync.dma_start(col_sb[:, :], col_d[:, :]).then_inc(s_ld, 16)
            sync.dma_start(rl_sb[:, :], rl_d[:, :]).then_inc(s_ld, 16)
            for L in range(16):
                if L >= 4:
                    sync.wait_ge(s_hmm, 10 * (L - 3))
                sync.dma_start(
                    xt_sb[:, L % 4, :, :, :],
                    xt_d[:, L * 2560:(L + 1) * 2560],
                ).then_inc(s_x[L % 4], 16)
            for s in range(NSLOT):
                sync.wait_ge(s_act, s + 1)
                if s >= 2:
                    sync.wait_ge(s_ost[s % 2], 16 * (s // 2))
                sync.dma_start(
                    o_d[s * 128:(s + 1) * 128, :], o_sb[:, s % 2, :]
                ).then_inc(s_ost[s % 2], 16)

        @block.gpsimd
        def _(gpsimd):
            gpsimd.wait_ge(s_ld, 48)
            for g in range(ng):
                st = gates[g]
                gpsimd.wait_ge(s_hst[0], 16 * ((st + 1) // 2))
                if st >= 2:
                    gpsimd.wait_ge(s_hst[1], 16 * (st // 2))
                if g >= 4:
                    gpsimd.wait_ge(s_smm, mm_after[CPG * (g - 3) - 1])
                gpsimd.dma_gather(
                    val_eb[:, g % 4, :, :],
                    h_d[0:gates[g] * 2048, :],
                    col_sb[:, g * (NIDX // 16):(g + 1) * (NIDX // 16)],
                    NIDX,
                    NIDX,
                    128,
                ).then_inc(s_gat[g % 4], 16)

        @block.tensor
        def _(tensor):
            tensor.wait_ge(s_ld, 48)
            # phase A: h tile i = xt_i^T @ W, four tiles per PSUM bank
            for i in range(NTP):
                L = i // 10
                if i % 10 == 0:
                    tensor.wait_ge(s_x[L % 4], 16 * (L // 4 + 1))
                i4, q = i // 4, i % 4
                if q == 0 and i4 >= 4:
                    tensor.wait_ge(s_hcp, i4 - 3)
                tensor.matmul(
                    pha[i4 % 4][:, q * 128:(q + 1) * 128],
                    xt_sb[:, L % 4, i % 10, 0, :],
                    cst_sb[:, 0, :],
                    start=True, stop=False,
                )
                tensor.matmul(
                    pha[i4 % 4][:, q * 128:(q + 1) * 128],
                    xt_sb[:, L % 4, i % 10, 1, :],
                    cst_sb[:, 1, :],
                    start=False, stop=True,
                ).then_inc(s_hmm, 1)
            # phase B: segment-sum straight off each gathered slab
            for k in range(ng):
                tensor.wait_ge(s_gat[k % 4], 16 * (k // 4 + 1))
                for jj in range(CPG * k, CPG * k + CPG):
                    if jj >= nch:
                        break
                    s, c = chunk_slot[jj]
                    tensor.wait_ge(s_s, jj + 1)
                    if c == 0 and s >= 4:
                        tensor.wait_ge(s_act, s - 3)
                    tensor.matmul(
                        po[s % 4][:, 0:128],
                        s_sb[:, jj % 16, :],
                        val_eb[:, k % 4, jj % CPG, :],
                        start=(c == 0), stop=False,
                    ).then_inc(s_smm, 1)
                    if c == pbs[s] - 1:
                        tensor.matmul(
                            po[s % 4][:, 0:128],
                            cst_sb[:, 3, :],
                            cst_sb[:, 4, :],
                            start=False, stop=True,
                        ).then_inc(s_smm, 1)

        @block.vector
        def _(vector):
            vector.wait_ge(s_ld, 48)
            # phase A: PSUM fp32 -> SBUF fp16, 4 h tiles per copy
            for i4 in range(NTP // 4):
                vector.wait_ge(s_hmm, 4 * (i4 + 1))
                vector.tensor_copy(
                    h_sb[:, i4 * 512:(i4 + 1) * 512], pha[i4 % 4][:, 0:512]
                ).then_inc(s_hcp, 1)
            # phase B: one-hot tiles S[e, n] = (iota[n] == rowloc[e])
            for j in range(nch):
                if j >= 16:
                    vector.wait_ge(s_smm, mm_after[j - 16])
                vector.tensor_scalar(
                    s_sb[:, j % 16, :],
                    cst_sb[:, 2, :],
                    rl_sb[:, j:j + 1],
                    None,
                    mybir.AluOpType.is_equal,
                ).then_inc(s_s, 1)

        @block.scalar
        def _(scalar):
            # h DRAM stores on the otherwise-idle ACT hwdge queue
            for k in range(NST):
                scalar.wait_ge(s_hcp, 4 * (k + 1))
                if k >= 2:
                    scalar.wait_ge(s_hst[k % 2], 16 * (k // 2))
                scalar.dma_start(
                    h_d[k * 2048:(k + 1) * 2048, :].rearrange(
                        "(t p) f -> p t f", p=128
                    ),
                    h_sb[:, k * 2048:(k + 1) * 2048],
                ).then_inc(s_hst[k % 2], 16)
            for s in range(NSLOT):
                scalar.wait_ge(s_smm, mm_after[cum[s + 1] - 1])
                if s >= 2:
                    # o_sb slot s%2 (written by relu s-2) is read by store s-2
                    scalar.wait_ge(s_ost[s % 2], 16 * (s // 2))
                scalar.activation(
                    o_sb[:, s % 2, :], po[s % 4][:, 0:128],
                    mybir.ActivationFunctionType.Relu,
                ).then_inc(s_act, 1)

    nc.compile()
    return nc


def _run(x, edge_index, weight, bias, trace=False):
    common, per_core, slot_block, meta = _host_prep(x, edge_index, weight, bias)
    nc = _build_program(meta)
    in_maps = [dict(common, **per_core[c]) for c in range(NCORES)]
    res = run_bass_kernel_spmd(nc, in_maps, list(range(NCORES)), trace=trace)
    out = np.zeros((NBLK * 128, FOUT), np.float32)
    for c in range(NCORES):
        oc = np.asarray(res.results[c]["out"], np.float32)
        for s in range(NSLOT):
            b = slot_block[c, s]
            if b >= 0:
                out[b * 128:(b + 1) * 128] = oc[s * 128:(s + 1) * 128]
    return np.ascontiguousarray(out[:N_NODES]), res


def kernel(x, edge_index, weight, bias):
    out, _ = _run(x, edge_index, weight, bias, trace=False)
    return out


# revision 23
# speedup vs baseline: 1.8283x; 1.0094x over previous
"""GNN message-passing (graph convolution) kernel for 8 Trainium2 NeuronCores.

    out = relu(segment_sum(h[col], row) + bias),  h = x @ W

Strategy (dst-block sharding -- no collectives needed):
  * Host sorts edges by destination node and buckets them into 157 blocks of
    128 dst nodes.  Blocks are sorted by edge count and dealt snake-wise into
    20 slots x 8 cores so that slot s holds 8 similarly-sized blocks; the
    per-slot chunk count pb_s = max ceil(cnt/128) over its blocks is a program
    constant shared by all cores (SPMD), minimizing padding.
  * Phase A (per core, replicated): h = x @ W on the PE in fp16 (PSUM fp32
    accumulate).  x is shipped pre-transposed [kk, tile, k, n] so the whole
    10.5MB loads in 16 large DMAs; h stays IN SBUF [128, 160*128] fp16 --
    never written to DRAM.
  * Phase B: SWDGE dma_gather with SBUF source (transpose=True, tokens=128)
    fetches 2048 edge rows per gather into valT [128f, 2048e]; the PE
    transposes each 128-edge chunk back to edge-major via an identity matmul
    (4 chunks share one PSUM bank), DVE/ACT copy the bank to SBUF, the DVE
    builds one-hot tiles S[e,n] = (iota == rowloc), and the PE accumulates
    out_slot += S^T @ val over all chunks of the slot in PSUM fp32 -- an
    exact segment-sum.  The bias is folded in as one extra matmul per slot
    with constant operands (identity x bias-broadcast): no gather, no DVE.
  * ACT applies ReLU PSUM->SBUF fp16; output stores are batched 2 blocks per
    DMA.  The host scatters block rows back to their original positions.

Numerics: fp16 operands with fp32 accumulation everywhere; the one-hot and
transpose matmuls are exact, so the only error is fp16 rounding of x, W, h
and the output (~1e-3 relative).
"""

import sys

import numpy as np

sys.path.insert(0, "/opt/trn_rl_repo")

import concourse.bacc as bacc  # noqa: E402
import concourse.mybir as mybir  # noqa: E402
from concourse.bass_utils import run_bass_kernel_spmd  # noqa: E402

N_NODES = 20000
FIN = 256
FOUT = 128
N_EDGES = 640000

NT = 157                 # real node tiles of 128
NTP = 160                # padded tiles (x zero-padded) -> h_sb rows 20480
NPAD = NTP * 128
NBLK = 157               # dst blocks of 128 nodes
NCORES = 8
NSLOT = 20               # block slots per core (slot 19: 5 real + 3 dummy)
NIDX = 1024              # idxs per dma_gather (8 chunks)
CPG = NIDX // 128        # chunks per gather
SCRATCH = 16384          # stock SWDGE ring (1024 descriptors)

FP16 = mybir.dt.float16
FP32 = mybir.dt.float32
I16 = mybir.dt.int16


def _plan(edge_index):
    """Sort/bucket edges; derive the SPMD-uniform slot structure."""
    row = np.asarray(edge_index[0]).astype(np.int64)
    col = np.asarray(edge_index[1]).astype(np.int64)
    order = np.argsort(row, kind="stable")
    rs = row[order].astype(np.int32)
    cs = col[order].astype(np.int32)

    blk = rs >> 7
    counts = np.bincount(blk, minlength=NBLK)
    starts = np.concatenate([[0], np.cumsum(counts)])

    big_first = np.argsort(counts, kind="stable")[::-1]  # block ids by size desc
    pbs = []
    slot_block = np.full((NCORES, NSLOT), -1, np.int64)
    for s in range(NSLOT):
        grp = big_first[s * NCORES:(s + 1) * NCORES]
        pbs.append(int(((counts[grp] + 127) // 128).max()))
        for c, b in enumerate(grp):
            slot_block[c, s] = b
    cum = np.concatenate([[0], np.cumsum(pbs)])
    nch = int(cum[-1])
    cpg = NIDX // 128
    ng = (nch + cpg - 1) // cpg
    nchp = ng * cpg
    return rs, cs, starts, slot_block, pbs, cum, nch, nchp, ng


def _host_prep(x, edge_index, weight, bias):
    """Cast/retile operands; build per-core gather index / rowloc tables."""
    x = np.asarray(x, np.float32)
    weight = np.asarray(weight, np.float32)
    bias = np.asarray(bias, np.float32)

    rs, cs, starts, slot_block, pbs, cum, nch, nchp, ng = _plan(edge_index)

    xpad = np.zeros((NPAD, FIN), np.float32)
    xpad[:N_NODES] = x
    # xt[kk, tile*256 + k*128 + n] = x[tile*128 + n, k*128 + kk]
    xt = np.ascontiguousarray(
        xpad.reshape(NTP, 128, 2, 128)        # [tile, n, k, kk]
        .transpose(3, 0, 2, 1)                 # [kk, tile, k, n]
        .reshape(128, NTP * 2 * 128)
        .astype(np.float16)
    )
    # cst[:, 0:2, :] = W chunks; 2: iota; 3: identity; 4: bias broadcast
    cst = np.zeros((128, 5, 128), np.float16)
    cst[:, 0:2, :] = weight.reshape(2, 128, 128).transpose(1, 0, 2)
    cst[:, 2, :] = np.arange(128, dtype=np.float16)[None, :]
    cst[:, 3, :] = np.eye(128, dtype=np.float16)
    cst[:, 4, :] = bias.astype(np.float16)[None, :]

    gmax = np.zeros(ng, np.int64)
    col16 = np.zeros((NCORES, 128, ng * (NIDX // 16)), np.int16)
    rl32 = np.full((NCORES, 128, nchp), -1.0, np.float32)
    for c in range(NCORES):
        lin_col = np.zeros(nchp * 128, np.int32)
        lin_rl = np.full(nchp * 128, -1.0, np.float32)
        for s in range(NSLOT):
            b = slot_block[c, s]
            if b < 0:
                continue
            e0, e1 = int(starts[b]), int(starts[b + 1])
            k = e1 - e0
            j0 = int(cum[s]) * 128
            o = np.argsort(cs[e0:e1], kind="stable")
            lin_col[j0:j0 + k] = cs[e0:e1][o]
            lin_rl[j0:j0 + k] = (rs[e0:e1] - b * 128)[o]
        # SWDGE idx layout: idx i -> partition i%16, column i//16 (x8 repl.)
        col16[c] = np.tile(
            lin_col.reshape(nchp * 128 // 16, 16).T.astype(np.int16), (8, 1)
        )
        rl32[c] = lin_rl.reshape(nchp, 128).T
        gmax = np.maximum(gmax, lin_col.reshape(ng, NIDX).max(axis=1))

    # per-gather h-frontier gate: h DRAM stores of 16 tiles (2048 rows)
    gates = [int(v) for v in (gmax // 2048 + 1)]
    meta = dict(
        pbs=pbs, cum=[int(v) for v in cum], nch=nch, nchp=nchp, ng=ng,
        gates=gates,
    )
    common = {"xt": xt, "cst": cst}
    per_core = [
        {"col": np.ascontiguousarray(col16[c]), "rl": np.ascontiguousarray(rl32[c])}
        for c in range(NCORES)
    ]
    return common, per_core, slot_block, meta


def _build_program(meta):
    pbs, cum = meta["pbs"], meta["cum"]
    nch, nchp, ng = meta["nch"], meta["nchp"], meta["ng"]
    gates = meta["gates"]
    chunk_slot = []                  # chunk j -> (slot, c)
    for s in range(NSLOT):
        for c in range(pbs[s]):
            chunk_slot.append((s, c))

    # cumulative segsum+bias matmul count after chunk j
    mm_after = []
    tot = 0
    for j in range(nch):
        s, c = chunk_slot[j]
        tot += 1
        if c == pbs[s] - 1:
            tot += 1
        mm_after.append(tot)

    NST = NTP * 128 // 2048          # h DRAM stores (16 tiles each)
    # emit h store k right after the x load that covers its tiles
    store_after_load = {}
    for k in range(NST):
        L = -(-(16 * (k + 1)) // 10) - 1
        store_after_load.setdefault(L, []).append(k)

    nc = bacc.Bacc("TRN2", dynamic_dma_scratch_size=SCRATCH)

    xt_d = nc.dram_tensor("xt", [128, NTP * 2 * 128], FP16, kind="ExternalInput")
    cst_d = nc.dram_tensor("cst", [128, 5, 128], FP16, kind="ExternalInput")
    col_d = nc.dram_tensor("col", [128, ng * (NIDX // 16)], I16, kind="ExternalInput")
    rl_d = nc.dram_tensor("rl", [128, nchp], FP32, kind="ExternalInput")
    h_d = nc.dram_tensor("hbuf", [NTP * 128, 128], FP16)
    o_d = nc.dram_tensor("out", [128, NSLOT * 128], FP16, kind="ExternalOutput")

    from contextlib import ExitStack

    with ExitStack() as es:
        pha = [es.enter_context(nc.psum_tensor(f"pha{k}", [128, 512], FP32)) for k in range(4)]
        po = [es.enter_context(nc.psum_tensor(f"po{k}", [128, 512], FP32)) for k in range(4)]
        xt_sb = es.enter_context(nc.sbuf_tensor("xt_sb", [128, 4, 10, 2, 128], FP16))
        cst_sb = es.enter_context(nc.sbuf_tensor("cst_sb", [128, 5, 128], FP16))
        h_sb = es.enter_context(nc.sbuf_tensor("h_sb", [128, NTP * 128], FP16))
        val_eb = es.enter_context(nc.sbuf_tensor("val_eb", [128, 4, CPG, 128], FP16))
        s_sb = es.enter_context(nc.sbuf_tensor("s_sb", [128, 16, 128], FP16))
        o_sb = es.enter_context(nc.sbuf_tensor("o_sb", [128, 2, 128], FP16))
        col_sb = es.enter_context(nc.sbuf_tensor("col_sb", [128, ng * (NIDX // 16)], I16))
        rl_sb = es.enter_context(nc.sbuf_tensor("rl_sb", [128, nchp], FP32))

        s_x = [es.enter_context(nc.semaphore(f"s_x{k}")) for k in range(4)]
        s_ld = es.enter_context(nc.semaphore("s_ld"))
        s_hmm = es.enter_context(nc.semaphore("s_hmm"))
        s_hcp = es.enter_context(nc.semaphore("s_hcp"))
        s_hst = [es.enter_context(nc.semaphore(f"s_hst{k}")) for k in range(2)]
        s_gat = [es.enter_context(nc.semaphore(f"s_gat{k}")) for k in range(4)]
        s_s = es.enter_context(nc.semaphore("s_s"))
        s_smm = es.enter_context(nc.semaphore("s_smm"))
        s_act = es.enter_context(nc.semaphore("s_act"))
        s_ost = [es.enter_context(nc.semaphore(f"s_ost{k}")) for k in range(2)]
        block = es.enter_context(nc.Block())

        @block.sync
        def _(sync):
            sync.dma_start(cst_sb[:, :, :], cst_d[:, :, :]).then_inc(s_ld, 16)
            sync.dma_start(col_sb[:, :], col_d[:, :]).then_inc(s_ld, 16)
            sync.dma_start(rl_sb[:, :], rl_d[:, :]).then_inc(s_ld, 16)
            for L in range(16):
                if L >= 4:
                    sync.wait_ge(s_hmm, 10 * (L - 3))
                sync.dma_start(
                    xt_sb[:, L % 4, :, :, :],
                    xt_d[:, L * 2560:(L + 1) * 2560],
                ).then_inc(s_x[L % 4], 16)
            for k in range(NSLOT // 2):
                sync.wait_ge(s_act, 2 * (k + 1))
                if k >= 2:
                    sync.wait_ge(s_ost[k % 2], 16 * (k // 2))
                sync.dma_start(
                    o_d[:, k * 256:(k + 1) * 256], o_sb[:, :, :]
                ).then_inc(s_ost[k % 2], 16)

        @block.gpsimd
        def _(gpsimd):
            gpsimd.wait_ge(s_ld, 48)
            for g in range(ng):
                st = gates[g]
                gpsimd.wait_ge(s_hst[0], 16 * ((st + 1) // 2))
                if st >= 2:
                    gpsimd.wait_ge(s_hst[1], 16 * (st // 2))
                if g >= 4:
                    gpsimd.wait_ge(s_smm, mm_after[CPG * (g - 3) - 1])
                gpsimd.dma_gather(
                    val_eb[:, g % 4, :, :],
                    h_d[0:gates[g] * 2048, :],
                    col_sb[:, g * (NIDX // 16):(g + 1) * (NIDX // 16)],
                    NIDX,
                    NIDX,
                    128,
                ).then_inc(s_gat[g % 4], 16)

        @block.tensor
        def _(tensor):
            tensor.wait_ge(s_ld, 48)
            # phase A: h tile i = xt_i^T @ W, four tiles per PSUM bank
            for i in range(NTP):
                L = i // 10
                if i % 10 == 0:
                    tensor.wait_ge(s_x[L % 4], 16 * (L // 4 + 1))
                i4, q = i // 4, i % 4
                if q == 0 and i4 >= 4:
                    tensor.wait_ge(s_hcp, i4 - 3)
                tensor.matmul(
                    pha[i4 % 4][:, q * 128:(q + 1) * 128],
                    xt_sb[:, L % 4, i % 10, 0, :],
                    cst_sb[:, 0, :],
                    start=True, stop=False,
                )
                tensor.matmul(
                    pha[i4 % 4][:, q * 128:(q + 1) * 128],
                    xt_sb[:, L % 4, i % 10, 1, :],
                    cst_sb[:, 1, :],
                    start=False, stop=True,
                ).then_inc(s_hmm, 1)
            # phase B: segment-sum straight off each gathered slab
            for k in range(ng):
                tensor.wait_ge(s_gat[k % 4], 16 * (k // 4 + 1))
                for jj in range(CPG * k, CPG * k + CPG):
                    if jj >= nch:
                        break
                    s, c = chunk_slot[jj]
                    tensor.wait_ge(s_s, jj + 1)
                    if c == 0 and s >= 4:
                        tensor.wait_ge(s_act, s - 3)
                    tensor.matmul(
                        po[s % 4][:, 0:128],
                        s_sb[:, jj % 16, :],
                        val_eb[:, k % 4, jj % CPG, :],
                        start=(c == 0), stop=False,
                    ).then_inc(s_smm, 1)
                    if c == pbs[s] - 1:
                        tensor.matmul(
                            po[s % 4][:, 0:128],
                            cst_sb[:, 3, :],
                            cst_sb[:, 4, :],
                            start=False, stop=True,
                        ).then_inc(s_smm, 1)

        @block.vector
        def _(vector):
            vector.wait_ge(s_ld, 48)
            # phase A: PSUM fp32 -> SBUF fp16, 4 h tiles per copy
            for i4 in range(NTP // 4):
                vector.wait_ge(s_hmm, 4 * (i4 + 1))
                vector.tensor_copy(
                    h_sb[:, i4 * 512:(i4 + 1) * 512], pha[i4 % 4][:, 0:512]
                ).then_inc(s_hcp, 1)
            # phase B: one-hot tiles S[e, n] = (iota[n] == rowloc[e])
            for j in range(nch):
                if j >= 16:
                    vector.wait_ge(s_smm, mm_after[j - 16])
                vector.tensor_scalar(
                    s_sb[:, j % 16, :],
                    cst_sb[:, 2, :],
                    rl_sb[:, j:j + 1],
                    None,
                    mybir.AluOpType.is_equal,
                ).then_inc(s_s, 1)

        @block.scalar
        def _(scalar):
            # h DRAM stores on the otherwise-idle ACT hwdge queue
            for k in range(NST):
                scalar.wait_ge(s_hcp, 4 * (k + 1))
                if k >= 2:
                    scalar.wait_ge(s_hst[k % 2], 16 * (k // 2))
                scalar.dma_start(
                    h_d[k * 2048:(k + 1) * 2048, :].rearrange(
                        "(t p) f -> p t f", p=128
                    ),
                    h_sb[:, k * 2048:(k + 1) * 2048],
                ).then_inc(s_hst[k % 2], 16)
            for s in range(NSLOT):
                scalar.wait_ge(s_smm, mm_after[cum[s + 1] - 1])
                if s >= 2:
                    # o_sb slot s%2 (written by relu s-2) is read by store (s-2)//2
                    k0 = (s - 2) // 2
                    scalar.wait_ge(s_ost[k0 % 2], 16 * (k0 // 2 + 1))
                scalar.activation(
                    o_sb[:, s % 2, :], po[s % 4][:, 0:128],
                    mybir.ActivationFunctionType.Relu,
                ).then_inc(s_act, 1)

    nc.compile()
    return nc


def _decode_out(oc):
    """[128, NSLOT*128] partition-major -> [NSLOT*128 rows, 128] fp32."""
    return np.ascontiguousarray(
        oc.reshape(128, NSLOT, 128).transpose(1, 0, 2).reshape(NSLOT * 128, 128)
    ).astype(np.float32)


def _run(x, edge_index, weight, bias, trace=False):
    common, per_core, slot_block, meta = _host_prep(x, edge_index, weight, bias)
    nc = _build_program(meta)
    in_maps = [dict(common, **per_core[c]) for c in range(NCORES)]
    res = run_bass_kernel_spmd(nc, in_maps, list(range(NCORES)), trace=trace)
    out = np.zeros((NBLK * 128, FOUT), np.float32)
    for c in range(NCORES):
        oc = _decode_out(np.asarray(res.results[c]["out"]))
        for s in range(NSLOT):
            b = slot_block[c, s]
            if b >= 0:
                out[b * 128:(b + 1) * 128] = oc[s * 128:(s + 1) * 128]
    return np.ascontiguousarray(out[:N_NODES]), res


def kernel(x, edge_index, weight, bias):
    out, _ = _run(x, edge_index, weight, bias, trace=False)
    return out


# revision 30
# speedup vs baseline: 1.8571x; 1.0158x over previous
"""GNN message-passing (graph convolution) kernel for 8 Trainium2 NeuronCores.

    out = relu(segment_sum(h[col], row) + bias),  h = x @ W

Strategy (dst-block sharding -- no collectives needed):
  * Host sorts edges by destination node and buckets them into 157 blocks of
    128 dst nodes.  Blocks are sorted by edge count and dealt snake-wise into
    20 slots x 8 cores so that slot s holds 8 similarly-sized blocks; the
    per-slot chunk count pb_s = max ceil(cnt/128) over its blocks is a program
    constant shared by all cores (SPMD), minimizing padding.
  * Phase A (per core, replicated): h = x @ W on the PE in fp16 (PSUM fp32
    accumulate).  x is shipped pre-transposed [kk, tile, k, n] so the whole
    10.5MB loads in 16 large DMAs; h stays IN SBUF [128, 160*128] fp16 --
    never written to DRAM.
  * Phase B: SWDGE dma_gather with SBUF source (transpose=True, tokens=128)
    fetches 2048 edge rows per gather into valT [128f, 2048e]; the PE
    transposes each 128-edge chunk back to edge-major via an identity matmul
    (4 chunks share one PSUM bank), DVE/ACT copy the bank to SBUF, the DVE
    builds one-hot tiles S[e,n] = (iota == rowloc), and the PE accumulates
    out_slot += S^T @ val over all chunks of the slot in PSUM fp32 -- an
    exact segment-sum.  The bias is folded in as one extra matmul per slot
    with constant operands (identity x bias-broadcast): no gather, no DVE.
  * ACT applies ReLU PSUM->SBUF fp16; output stores are batched 2 blocks per
    DMA.  The host scatters block rows back to their original positions.

Numerics: fp16 operands with fp32 accumulation everywhere; the one-hot and
transpose matmuls are exact, so the only error is fp16 rounding of x, W, h
and the output (~1e-3 relative).
"""

import sys

import numpy as np

sys.path.insert(0, "/opt/trn_rl_repo")

import concourse.bacc as bacc  # noqa: E402
import concourse.mybir as mybir  # noqa: E402
from concourse.bass_utils import run_bass_kernel_spmd  # noqa: E402

N_NODES = 20000
FIN = 256
FOUT = 128
N_EDGES = 640000

NT = 157                 # real node tiles of 128
NTP = 160                # padded tiles (x zero-padded) -> h_sb rows 20480
NPAD = NTP * 128
NBLK = 157               # dst blocks of 128 nodes
NCORES = 8
NSLOT = 20               # block slots per core (slot 19: 5 real + 3 dummy)
NIDX = 1024              # idxs per dma_gather (8 chunks)
CPG = NIDX // 128        # chunks per gather
SCRATCH = 16384          # stock SWDGE ring (1024 descriptors)

FP16 = mybir.dt.float16
FP32 = mybir.dt.float32
I16 = mybir.dt.int16


def _plan(edge_index):
    """Sort/bucket edges; derive the SPMD-uniform slot structure."""
    row = np.asarray(edge_index[0]).astype(np.int64)
    col = np.asarray(edge_index[1]).astype(np.int64)
    order = np.argsort(row, kind="stable")
    rs = row[order].astype(np.int32)
    cs = col[order].astype(np.int32)

    blk = rs >> 7
    counts = np.bincount(blk, minlength=NBLK)
    starts = np.concatenate([[0], np.cumsum(counts)])

    big_first = np.argsort(counts, kind="stable")[::-1]  # block ids by size desc
    pbs = []
    slot_block = np.full((NCORES, NSLOT), -1, np.int64)
    for s in range(NSLOT):
        grp = big_first[s * NCORES:(s + 1) * NCORES]
        pbs.append(int(((counts[grp] + 127) // 128).max()))
        for c, b in enumerate(grp):
            slot_block[c, s] = b
    cum = np.concatenate([[0], np.cumsum(pbs)])
    nch = int(cum[-1])
    cpg = NIDX // 128
    ng = (nch + cpg - 1) // cpg
    nchp = ng * cpg
    return rs, cs, starts, slot_block, pbs, cum, nch, nchp, ng


def _host_prep(x, edge_index, weight, bias):
    """Cast/retile operands; build per-core gather index / rowloc tables."""
    x = np.asarray(x, np.float32)
    weight = np.asarray(weight, np.float32)
    bias = np.asarray(bias, np.float32)

    rs, cs, starts, slot_block, pbs, cum, nch, nchp, ng = _plan(edge_index)

    xpad = np.zeros((NPAD, FIN), np.float32)
    xpad[:N_NODES] = x
    # xt[kk, tile*256 + k*128 + n] = x[tile*128 + n, k*128 + kk]
    xt = np.ascontiguousarray(
        xpad.reshape(NTP, 128, 2, 128)        # [tile, n, k, kk]
        .transpose(3, 0, 2, 1)                 # [kk, tile, k, n]
        .reshape(128, NTP * 2 * 128)
        .astype(np.float16)
    )
    # cst[:, 0:2, :] = W chunks; 2: iota; 3: identity; 4: bias broadcast
    cst = np.zeros((128, 5, 128), np.float16)
    cst[:, 0:2, :] = weight.reshape(2, 128, 128).transpose(1, 0, 2)
    cst[:, 2, :] = np.arange(128, dtype=np.float16)[None, :]
    cst[:, 3, :] = np.eye(128, dtype=np.float16)
    cst[:, 4, :] = bias.astype(np.float16)[None, :]

    gmax = np.zeros(ng, np.int64)
    col16 = np.zeros((NCORES, 32, ng * (NIDX // 16)), np.int16)
    rl32 = np.full((NCORES, 128, nchp), -1.0, np.float32)
    for c in range(NCORES):
        lin_col = np.zeros(nchp * 128, np.int32)
        lin_rl = np.full(nchp * 128, -1.0, np.float32)
        for s in range(NSLOT):
            b = slot_block[c, s]
            if b < 0:
                continue
            e0, e1 = int(starts[b]), int(starts[b + 1])
            k = e1 - e0
            j0 = int(cum[s]) * 128
            o = np.argsort(cs[e0:e1], kind="stable")
            lin_col[j0:j0 + k] = cs[e0:e1][o]
            lin_rl[j0:j0 + k] = (rs[e0:e1] - b * 128)[o]
        # SWDGE idx layout: idx i -> partition i%16, column i//16 (x8 repl.)
        col16[c] = np.tile(
            lin_col.reshape(nchp * 128 // 16, 16).T.astype(np.int16), (2, 1)
        )
        rl32[c] = lin_rl.reshape(nchp, 128).T
        gmax = np.maximum(gmax, lin_col.reshape(ng, NIDX).max(axis=1))

    # per-gather h-frontier gate: h DRAM stores of 16 tiles (2048 rows)
    gates = [int(v) for v in (gmax // 2048 + 1)]
    meta = dict(
        pbs=pbs, cum=[int(v) for v in cum], nch=nch, nchp=nchp, ng=ng,
        gates=gates,
    )
    common = {"xt": xt, "cst": cst}
    per_core = [
        {"col": np.ascontiguousarray(col16[c]), "rl": np.ascontiguousarray(rl32[c])}
        for c in range(NCORES)
    ]
    return common, per_core, slot_block, meta


def _build_program(meta):
    pbs, cum = meta["pbs"], meta["cum"]
    nch, nchp, ng = meta["nch"], meta["nchp"], meta["ng"]
    gates = meta["gates"]
    chunk_slot = []                  # chunk j -> (slot, c)
    for s in range(NSLOT):
        for c in range(pbs[s]):
            chunk_slot.append((s, c))

    # cumulative segsum+bias matmul count after chunk j
    mm_after = []
    tot = 0
    for j in range(nch):
        s, c = chunk_slot[j]
        tot += 1
        if c == pbs[s] - 1:
            tot += 1
        mm_after.append(tot)

    NST = NTP * 128 // 2048          # h DRAM stores (16 tiles each)
    # emit h store k right after the x load that covers its tiles
    store_after_load = {}
    for k in range(NST):
        L = -(-(16 * (k + 1)) // 10) - 1
        store_after_load.setdefault(L, []).append(k)

    nc = bacc.Bacc("TRN2", dynamic_dma_scratch_size=SCRATCH)

    xt_d = nc.dram_tensor("xt", [128, NTP * 2 * 128], FP16, kind="ExternalInput")
    cst_d = nc.dram_tensor("cst", [128, 5, 128], FP16, kind="ExternalInput")
    col_d = nc.dram_tensor("col", [32, ng * (NIDX // 16)], I16, kind="ExternalInput")
    rl_d = nc.dram_tensor("rl", [128, nchp], FP32, kind="ExternalInput")
    h_d = nc.dram_tensor("hbuf", [NTP * 128, 128], FP16)
    o_d = nc.dram_tensor("out", [128, NSLOT * 128], FP16, kind="ExternalOutput")

    from contextlib import ExitStack

    with ExitStack() as es:
        pha = [es.enter_context(nc.psum_tensor(f"pha{k}", [128, 512], FP32)) for k in range(4)]
        po = [es.enter_context(nc.psum_tensor(f"po{k}", [128, 512], FP32)) for k in range(4)]
        xt_sb = es.enter_context(nc.sbuf_tensor("xt_sb", [128, 8, 10, 2, 128], FP16))
        cst_sb = es.enter_context(nc.sbuf_tensor("cst_sb", [128, 5, 128], FP16))
        h_sb = es.enter_context(nc.sbuf_tensor("h_sb", [128, NTP * 128], FP16))
        val_eb = es.enter_context(nc.sbuf_tensor("val_eb", [128, 4, CPG, 128], FP16))
        s_sb = es.enter_context(nc.sbuf_tensor("s_sb", [128, 16, 128], FP16))
        o_sb = es.enter_context(nc.sbuf_tensor("o_sb", [128, 2, 128], FP16))
        col_sb = es.enter_context(nc.sbuf_tensor("col_sb", [128, ng * (NIDX // 16)], I16))
        rl_sb = es.enter_context(nc.sbuf_tensor("rl_sb", [128, nchp], FP32))

        s_x = [es.enter_context(nc.semaphore(f"s_x{k}")) for k in range(8)]
        s_ld = es.enter_context(nc.semaphore("s_ld"))
        s_msk = es.enter_context(nc.semaphore("s_msk"))
        s_hmm = es.enter_context(nc.semaphore("s_hmm"))
        s_hcp = es.enter_context(nc.semaphore("s_hcp"))
        s_hst = [es.enter_context(nc.semaphore(f"s_hst{k}")) for k in range(2)]
        s_gat = [es.enter_context(nc.semaphore(f"s_gat{k}")) for k in range(4)]
        s_s = es.enter_context(nc.semaphore("s_s"))
        s_smm = es.enter_context(nc.semaphore("s_smm"))
        s_act = es.enter_context(nc.semaphore("s_act"))
        s_ost = [es.enter_context(nc.semaphore(f"s_ost{k}")) for k in range(2)]
        block = es.enter_context(nc.Block())

        @block.sync
        def _(sync):
            sync.dma_start(cst_sb[:, :, :], cst_d[:, :, :]).then_inc(s_ld, 16)
            sync.dma_start(col_sb[0:32, :], col_d[:, :]).then_inc(s_ld, 16)
            sync.dma_start(rl_sb[:, :], rl_d[:, :]).then_inc(s_ld, 16)
            for L in range(16):
                if L >= 8:
                    sync.wait_ge(s_hmm, 10 * (L - 7))
                sync.dma_start(
                    xt_sb[:, L % 8, :, :, :],
                    xt_d[:, L * 2560:(L + 1) * 2560],
                ).then_inc(s_x[L % 8], 16)
            for k in range(NSLOT // 2):
                sync.wait_ge(s_act, 2 * (k + 1))
                if k >= 2:
                    sync.wait_ge(s_ost[k % 2], 16 * (k // 2))
                sync.dma_start(
                    o_d[:, k * 256:(k + 1) * 256], o_sb[:, :, :]
                ).then_inc(s_ost[k % 2], 16)

        @block.gpsimd
        def _(gpsimd):
            gpsimd.wait_ge(s_ld, 48)
            gpsimd.wait_ge(s_msk, 2)
            for g in range(ng):
                st = gates[g]
                gpsimd.wait_ge(s_hst[0], 16 * ((st + 1) // 2))
                if st >= 2:
                    gpsimd.wait_ge(s_hst[1], 16 * (st // 2))
                if g >= 4:
                    gpsimd.wait_ge(s_smm, mm_after[CPG * (g - 3) - 1])
                gpsimd.dma_gather(
                    val_eb[:, g % 4, :, :],
                    h_d[0:gates[g] * 2048, :],
                    col_sb[:, g * (NIDX // 16):(g + 1) * (NIDX // 16)],
                    NIDX,
                    NIDX,
                    128,
                ).then_inc(s_gat[g % 4], 16)

        @block.tensor
        def _(tensor):
            tensor.wait_ge(s_ld, 48)
            # phase A: h tile i = xt_i^T @ W, four tiles per PSUM bank
            for i in range(NTP):
                L = i // 10
                if i % 10 == 0:
                    tensor.wait_ge(s_x[L % 8], 16 * (L // 8 + 1))
                i4, q = i // 4, i % 4
                if q == 0 and i4 >= 4:
                    tensor.wait_ge(s_hcp, i4 - 3)
                tensor.matmul(
                    pha[i4 % 4][:, q * 128:(q + 1) * 128],
                    xt_sb[:, L % 8, i % 10, 0, :],
                    cst_sb[:, 0, :],
                    start=True, stop=False,
                )
                tensor.matmul(
                    pha[i4 % 4][:, q * 128:(q + 1) * 128],
                    xt_sb[:, L % 8, i % 10, 1, :],
                    cst_sb[:, 1, :],
                    start=False, stop=True,
                ).then_inc(s_hmm, 1)
            # phase B: segment-sum straight off each gathered slab
            for k in range(ng):
                tensor.wait_ge(s_gat[k % 4], 16 * (k // 4 + 1))
                for jj in range(CPG * k, CPG * k + CPG):
                    if jj >= nch:
                        break
                    s, c = chunk_slot[jj]
                    tensor.wait_ge(s_s, jj + 1)
                    if c == 0 and s >= 4:
                        tensor.wait_ge(s_act, s - 3)
                    tensor.matmul(
                        po[s % 4][:, 0:128],
                        s_sb[:, jj % 16, :],
                        val_eb[:, k % 4, jj % CPG, :],
                        start=(c == 0), stop=False,
                    ).then_inc(s_smm, 1)
                    if c == pbs[s] - 1:
                        tensor.matmul(
                            po[s % 4][:, 0:128],
                            cst_sb[:, 3, :],
                            cst_sb[:, 4, :],
                            start=False, stop=True,
                        ).then_inc(s_smm, 1)

        @block.vector
        def _(vector):
            # top idx partitions are never read by SWDGE; zero them so the
            # interp's bounds assert sees valid values
            vector.memset(col_sb[32:64, :], 0).then_inc(s_msk, 1)
            vector.memset(col_sb[64:128, :], 0).then_inc(s_msk, 1)
            vector.wait_ge(s_ld, 48)
            # phase A: PSUM fp32 -> SBUF fp16, 4 h tiles per copy
            for i4 in range(NTP // 4):
                vector.wait_ge(s_hmm, 4 * (i4 + 1))
                vector.tensor_copy(
                    h_sb[:, i4 * 512:(i4 + 1) * 512], pha[i4 % 4][:, 0:512]
                ).then_inc(s_hcp, 1)
            # phase B: one-hot tiles S[e, n] = (iota[n] == rowloc[e])
            for j in range(nch):
                if j >= 16:
                    vector.wait_ge(s_smm, mm_after[j - 16])
                vector.tensor_scalar(
                    s_sb[:, j % 16, :],
                    cst_sb[:, 2, :],
                    rl_sb[:, j:j + 1],
                    None,
                    mybir.AluOpType.is_equal,
                ).then_inc(s_s, 1)

        @block.scalar
        def _(scalar):
            # h DRAM stores on the otherwise-idle ACT hwdge queue
            for k in range(NST):
                scalar.wait_ge(s_hcp, 4 * (k + 1))
                if k >= 2:
                    scalar.wait_ge(s_hst[k % 2], 16 * (k // 2))
                scalar.dma_start(
                    h_d[k * 2048:(k + 1) * 2048, :].rearrange(
                        "(t p) f -> p t f", p=128
                    ),
                    h_sb[:, k * 2048:(k + 1) * 2048],
                ).then_inc(s_hst[k % 2], 16)
            for s in range(NSLOT):
                scalar.wait_ge(s_smm, mm_after[cum[s + 1] - 1])
                if s >= 2:
                    # o_sb slot s%2 (written by relu s-2) is read by store (s-2)//2
                    k0 = (s - 2) // 2
                    scalar.wait_ge(s_ost[k0 % 2], 16 * (k0 // 2 + 1))
                scalar.activation(
                    o_sb[:, s % 2, :], po[s % 4][:, 0:128],
                    mybir.ActivationFunctionType.Relu,
                ).then_inc(s_act, 1)

    nc.compile()
    return nc


def _decode_out(oc):
    """[128, NSLOT*128] partition-major -> [NSLOT*128 rows, 128] fp32."""
    return np.ascontiguousarray(
        oc.reshape(128, NSLOT, 128).transpose(1, 0, 2).reshape(NSLOT * 128, 128)
    ).astype(np.float32)


def _run(x, edge_index, weight, bias, trace=False):
    common, per_core, slot_block, meta = _host_prep(x, edge_index, weight, bias)
    nc = _build_program(meta)
    in_maps = [dict(common, **per_core[c]) for c in range(NCORES)]
    res = run_bass_kernel_spmd(nc, in_maps, list(range(NCORES)), trace=trace)
    out = np.zeros((NBLK * 128, FOUT), np.float32)
    for c in range(NCORES):
        oc = _decode_out(np.asarray(res.results[c]["out"]))
        for s in range(NSLOT):
            b = slot_block[c, s]
            if b >= 0:
                out[b * 128:(b + 1) * 128] = oc[s * 128:(s + 1) * 128]
    return np.ascontiguousarray(out[:N_NODES]), res


def kernel(x, edge_index, weight, bias):
    out, _ = _run(x, edge_index, weight, bias, trace=False)
    return out


# revision 32
# speedup vs baseline: 1.8926x; 1.0191x over previous
"""GNN message-passing (graph convolution) kernel for 8 Trainium2 NeuronCores.

    out = relu(segment_sum(h[col], row) + bias),  h = x @ W

Strategy (dst-block sharding -- no collectives needed):
  * Host sorts edges by destination node and buckets them into 157 blocks of
    128 dst nodes.  Blocks are sorted by edge count and dealt snake-wise into
    20 slots x 8 cores so that slot s holds 8 similarly-sized blocks; the
    per-slot chunk count pb_s = max ceil(cnt/128) over its blocks is a program
    constant shared by all cores (SPMD), minimizing padding.
  * Phase A (per core, replicated): h = x @ W on the PE in fp16 (PSUM fp32
    accumulate).  x is shipped pre-transposed [kk, tile, k, n] so the whole
    10.5MB loads in 16 large DMAs; h stays IN SBUF [128, 160*128] fp16 --
    never written to DRAM.
  * Phase B: SWDGE dma_gather with SBUF source (transpose=True, tokens=128)
    fetches 2048 edge rows per gather into valT [128f, 2048e]; the PE
    transposes each 128-edge chunk back to edge-major via an identity matmul
    (4 chunks share one PSUM bank), DVE/ACT copy the bank to SBUF, the DVE
    builds one-hot tiles S[e,n] = (iota == rowloc), and the PE accumulates
    out_slot += S^T @ val over all chunks of the slot in PSUM fp32 -- an
    exact segment-sum.  The bias is folded in as one extra matmul per slot
    with constant operands (identity x bias-broadcast): no gather, no DVE.
  * ACT applies ReLU PSUM->SBUF fp16; output stores are batched 2 blocks per
    DMA.  The host scatters block rows back to their original positions.

Numerics: fp16 operands with fp32 accumulation everywhere; the one-hot and
transpose matmuls are exact, so the only error is fp16 rounding of x, W, h
and the output (~1e-3 relative).
"""

import sys

import numpy as np

sys.path.insert(0, "/opt/trn_rl_repo")

import concourse.bacc as bacc  # noqa: E402
import concourse.mybir as mybir  # noqa: E402
from concourse.bass_utils import run_bass_kernel_spmd  # noqa: E402

N_NODES = 20000
FIN = 256
FOUT = 128
N_EDGES = 640000

NT = 157                 # real node tiles of 128
NTP = 157                # no tile padding -- h rows 20096
NPAD = NTP * 128
NBLK = 157               # dst blocks of 128 nodes
NCORES = 8
NSLOT = 20               # block slots per core (slot 19: 5 real + 3 dummy)
NIDX = 1024              # idxs per dma_gather (8 chunks)
CPG = NIDX // 128        # chunks per gather
SCRATCH = 16384          # stock SWDGE ring (1024 descriptors)

FP16 = mybir.dt.float16
FP32 = mybir.dt.float32
I16 = mybir.dt.int16


def _plan(edge_index):
    """Sort/bucket edges; derive the SPMD-uniform slot structure."""
    row = np.asarray(edge_index[0]).astype(np.int64)
    col = np.asarray(edge_index[1]).astype(np.int64)
    order = np.argsort(row, kind="stable")
    rs = row[order].astype(np.int32)
    cs = col[order].astype(np.int32)

    blk = rs >> 7
    counts = np.bincount(blk, minlength=NBLK)
    starts = np.concatenate([[0], np.cumsum(counts)])

    big_first = np.argsort(counts, kind="stable")[::-1]  # block ids by size desc
    pbs = []
    slot_block = np.full((NCORES, NSLOT), -1, np.int64)
    for s in range(NSLOT):
        grp = big_first[s * NCORES:(s + 1) * NCORES]
        pbs.append(int(((counts[grp] + 127) // 128).max()))
        for c, b in enumerate(grp):
            slot_block[c, s] = b
    cum = np.concatenate([[0], np.cumsum(pbs)])
    nch = int(cum[-1])
    cpg = NIDX // 128
    ng = (nch + cpg - 1) // cpg
    nchp = ng * cpg
    return rs, cs, starts, slot_block, pbs, cum, nch, nchp, ng


def _host_prep(x, edge_index, weight, bias):
    """Cast/retile operands; build per-core gather index / rowloc tables."""
    x = np.asarray(x, np.float32)
    weight = np.asarray(weight, np.float32)
    bias = np.asarray(bias, np.float32)

    rs, cs, starts, slot_block, pbs, cum, nch, nchp, ng = _plan(edge_index)

    xpad = np.zeros((NPAD, FIN), np.float32)
    xpad[:N_NODES] = x
    # xt[kk, tile*256 + k*128 + n] = x[tile*128 + n, k*128 + kk]
    xt = np.ascontiguousarray(
        xpad.reshape(NTP, 128, 2, 128)        # [tile, n, k, kk]
        .transpose(3, 0, 2, 1)                 # [kk, tile, k, n]
        .reshape(128, NTP * 2 * 128)
        .astype(np.float16)
    )
    # cst[:, 0:2, :] = W chunks; 2: iota; 3: identity; 4: bias broadcast
    cst = np.zeros((128, 5, 128), np.float16)
    cst[:, 0:2, :] = weight.reshape(2, 128, 128).transpose(1, 0, 2)
    cst[:, 2, :] = np.arange(128, dtype=np.float16)[None, :]
    cst[:, 3, :] = np.eye(128, dtype=np.float16)
    cst[:, 4, :] = bias.astype(np.float16)[None, :]

    gmax = np.zeros(ng, np.int64)
    col16 = np.zeros((NCORES, 32, ng * (NIDX // 16)), np.int16)
    rl32 = np.full((NCORES, 128, nchp), -1.0, np.float32)
    for c in range(NCORES):
        lin_col = np.zeros(nchp * 128, np.int32)
        lin_rl = np.full(nchp * 128, -1.0, np.float32)
        for s in range(NSLOT):
            b = slot_block[c, s]
            if b < 0:
                continue
            e0, e1 = int(starts[b]), int(starts[b + 1])
            k = e1 - e0
            j0 = int(cum[s]) * 128
            o = np.argsort(cs[e0:e1], kind="stable")
            lin_col[j0:j0 + k] = cs[e0:e1][o]
            lin_rl[j0:j0 + k] = (rs[e0:e1] - b * 128)[o]
        # SWDGE idx layout: idx i -> partition i%16, column i//16 (x8 repl.)
        col16[c] = np.tile(
            lin_col.reshape(nchp * 128 // 16, 16).T.astype(np.int16), (2, 1)
        )
        rl32[c] = lin_rl.reshape(nchp, 128).T
        gmax = np.maximum(gmax, lin_col.reshape(ng, NIDX).max(axis=1))

    # per-gather h-frontier gate: h DRAM stores of 16 tiles (2048 rows)
    gates = [int(v) for v in (gmax // 2048 + 1)]
    meta = dict(
        pbs=pbs, cum=[int(v) for v in cum], nch=nch, nchp=nchp, ng=ng,
        gates=gates,
    )
    common = {"xt": xt, "cst": cst}
    per_core = [
        {"col": np.ascontiguousarray(col16[c]), "rl": np.ascontiguousarray(rl32[c])}
        for c in range(NCORES)
    ]
    return common, per_core, slot_block, meta


def _build_program(meta):
    pbs, cum = meta["pbs"], meta["cum"]
    nch, nchp, ng = meta["nch"], meta["nchp"], meta["ng"]
    gates = meta["gates"]
    chunk_slot = []                  # chunk j -> (slot, c)
    for s in range(NSLOT):
        for c in range(pbs[s]):
            chunk_slot.append((s, c))

    # cumulative segsum+bias matmul count after chunk j
    mm_after = []
    tot = 0
    for j in range(nch):
        s, c = chunk_slot[j]
        tot += 1
        if c == pbs[s] - 1:
            tot += 1
        mm_after.append(tot)

    NST = (NTP * 128 + 2047) // 2048  # h DRAM stores (16 tiles each)
    # emit h store k right after the x load that covers its tiles
    store_after_load = {}
    for k in range(NST):
        L = -(-(16 * (k + 1)) // 10) - 1
        store_after_load.setdefault(L, []).append(k)

    nc = bacc.Bacc("TRN2", dynamic_dma_scratch_size=SCRATCH)

    xt_d = nc.dram_tensor("xt", [128, NTP * 2 * 128], FP16, kind="ExternalInput")
    cst_d = nc.dram_tensor("cst", [128, 5, 128], FP16, kind="ExternalInput")
    col_d = nc.dram_tensor("col", [32, ng * (NIDX // 16)], I16, kind="ExternalInput")
    rl_d = nc.dram_tensor("rl", [128, nchp], FP32, kind="ExternalInput")
    h_d = nc.dram_tensor("hbuf", [NTP * 128, 128], FP16)
    o_d = nc.dram_tensor("out", [128, NSLOT * 128], FP16, kind="ExternalOutput")

    from contextlib import ExitStack

    with ExitStack() as es:
        pha = [es.enter_context(nc.psum_tensor(f"pha{k}", [128, 512], FP32)) for k in range(4)]
        po = [es.enter_context(nc.psum_tensor(f"po{k}", [128, 512], FP32)) for k in range(4)]
        xt_sb = es.enter_context(nc.sbuf_tensor("xt_sb", [128, 8, 10, 2, 128], FP16))
        cst_sb = es.enter_context(nc.sbuf_tensor("cst_sb", [128, 5, 128], FP16))
        h_sb = es.enter_context(nc.sbuf_tensor("h_sb", [128, NTP * 128], FP16))
        val_eb = es.enter_context(nc.sbuf_tensor("val_eb", [128, 4, CPG, 128], FP16))
        s_sb = es.enter_context(nc.sbuf_tensor("s_sb", [128, 16, 128], FP16))
        o_sb = es.enter_context(nc.sbuf_tensor("o_sb", [128, 2, 128], FP16))
        col_sb = es.enter_context(nc.sbuf_tensor("col_sb", [128, ng * (NIDX // 16)], I16))
        rl_sb = es.enter_context(nc.sbuf_tensor("rl_sb", [128, nchp], FP32))

        s_x = [es.enter_context(nc.semaphore(f"s_x{k}")) for k in range(8)]
        s_ld = es.enter_context(nc.semaphore("s_ld"))
        s_msk = es.enter_context(nc.semaphore("s_msk"))
        s_hmm = es.enter_context(nc.semaphore("s_hmm"))
        s_hcp = es.enter_context(nc.semaphore("s_hcp"))
        s_hst = [es.enter_context(nc.semaphore(f"s_hst{k}")) for k in range(2)]
        s_gat = [es.enter_context(nc.semaphore(f"s_gat{k}")) for k in range(4)]
        s_s = es.enter_context(nc.semaphore("s_s"))
        s_smm = es.enter_context(nc.semaphore("s_smm"))
        s_act = es.enter_context(nc.semaphore("s_act"))
        s_ost = [es.enter_context(nc.semaphore(f"s_ost{k}")) for k in range(2)]
        block = es.enter_context(nc.Block())

        @block.sync
        def _(sync):
            sync.dma_start(cst_sb[:, :, :], cst_d[:, :, :]).then_inc(s_ld, 16)
            sync.dma_start(col_sb[0:32, :], col_d[:, :]).then_inc(s_ld, 16)
            sync.dma_start(rl_sb[:, :], rl_d[:, :]).then_inc(s_ld, 16)
            for L in range(16):
                if L >= 8:
                    sync.wait_ge(s_hmm, 10 * (L - 7))
                nt = min(10, NTP - 10 * L)
                sync.dma_start(
                    xt_sb[:, L % 8, 0:nt, :, :],
                    xt_d[:, L * 2560:L * 2560 + nt * 256],
                ).then_inc(s_x[L % 8], 16)
            for k in range(NSLOT // 2):
                sync.wait_ge(s_act, 2 * (k + 1))
                if k >= 2:
                    sync.wait_ge(s_ost[k % 2], 16 * (k // 2))
                sync.dma_start(
                    o_d[:, k * 256:(k + 1) * 256], o_sb[:, :, :]
                ).then_inc(s_ost[k % 2], 16)

        @block.gpsimd
        def _(gpsimd):
            gpsimd.wait_ge(s_ld, 48)
            gpsimd.wait_ge(s_msk, 2)
            for g in range(ng):
                st = gates[g]
                gpsimd.wait_ge(s_hst[0], 16 * ((st + 1) // 2))
                if st >= 2:
                    gpsimd.wait_ge(s_hst[1], 16 * (st // 2))
                if g >= 4:
                    gpsimd.wait_ge(s_smm, mm_after[CPG * (g - 3) - 1])
                # last gather: only its real chunks
                nix = min(NIDX, (nch - CPG * g) * 128)
                gpsimd.dma_gather(
                    val_eb[:, g % 4, 0:nix // 128, :],
                    h_d[0:min(gates[g] * 2048, NTP * 128), :],
                    col_sb[:, g * (NIDX // 16):g * (NIDX // 16) + nix // 16],
                    nix,
                    nix,
                    128,
                ).then_inc(s_gat[g % 4], 16)

        @block.tensor
        def _(tensor):
            tensor.wait_ge(s_ld, 48)
            # phase A: h tile i = xt_i^T @ W, four tiles per PSUM bank
            for i in range(NTP):
                L = i // 10
                if i % 10 == 0:
                    tensor.wait_ge(s_x[L % 8], 16 * (L // 8 + 1))
                i4, q = i // 4, i % 4
                if q == 0 and i4 >= 4:
                    tensor.wait_ge(s_hcp, i4 - 3)
                tensor.matmul(
                    pha[i4 % 4][:, q * 128:(q + 1) * 128],
                    xt_sb[:, L % 8, i % 10, 0, :],
                    cst_sb[:, 0, :],
                    start=True, stop=False,
                )
                tensor.matmul(
                    pha[i4 % 4][:, q * 128:(q + 1) * 128],
                    xt_sb[:, L % 8, i % 10, 1, :],
                    cst_sb[:, 1, :],
                    start=False, stop=True,
                ).then_inc(s_hmm, 1)
            # phase B: segment-sum straight off each gathered slab
            for k in range(ng):
                tensor.wait_ge(s_gat[k % 4], 16 * (k // 4 + 1))
                for jj in range(CPG * k, CPG * k + CPG):
                    if jj >= nch:
                        break
                    s, c = chunk_slot[jj]
                    tensor.wait_ge(s_s, jj + 1)
                    if c == 0 and s >= 4:
                        tensor.wait_ge(s_act, s - 3)
                    tensor.matmul(
                        po[s % 4][:, 0:128],
                        s_sb[:, jj % 16, :],
                        val_eb[:, k % 4, jj % CPG, :],
                        start=(c == 0), stop=False,
                    ).then_inc(s_smm, 1)
                    if c == pbs[s] - 1:
                        tensor.matmul(
                            po[s % 4][:, 0:128],
                            cst_sb[:, 3, :],
                            cst_sb[:, 4, :],
                            start=False, stop=True,
                        ).then_inc(s_smm, 1)

        @block.vector
        def _(vector):
            # top idx partitions are never read by SWDGE; zero them so the
            # interp's bounds assert sees valid values
            vector.memset(col_sb[32:64, :], 0).then_inc(s_msk, 1)
            vector.memset(col_sb[64:128, :], 0).then_inc(s_msk, 1)
            vector.wait_ge(s_ld, 48)
            # phase A: PSUM fp32 -> SBUF fp16, 4 h tiles per copy
            for i4 in range((NTP + 3) // 4):
                nt = min(4, NTP - 4 * i4)
                vector.wait_ge(s_hmm, 4 * i4 + nt)
                vector.tensor_copy(
                    h_sb[:, i4 * 512:i4 * 512 + nt * 128],
                    pha[i4 % 4][:, 0:nt * 128],
                ).then_inc(s_hcp, 1)
            # phase B: one-hot tiles S[e, n] = (iota[n] == rowloc[e])
            for j in range(nch):
                if j >= 16:
                    vector.wait_ge(s_smm, mm_after[j - 16])
                vector.tensor_scalar(
                    s_sb[:, j % 16, :],
                    cst_sb[:, 2, :],
                    rl_sb[:, j:j + 1],
                    None,
                    mybir.AluOpType.is_equal,
                ).then_inc(s_s, 1)

        @block.scalar
        def _(scalar):
            # h DRAM stores on the otherwise-idle ACT hwdge queue
            for k in range(NST):
                rows = min(2048, NTP * 128 - k * 2048)
                scalar.wait_ge(s_hcp, min(4 * (k + 1), (NTP + 3) // 4))
                if k >= 2:
                    scalar.wait_ge(s_hst[k % 2], 16 * (k // 2))
                scalar.dma_start(
                    h_d[k * 2048:k * 2048 + rows, :].rearrange(
                        "(t p) f -> p t f", p=128
                    ),
                    h_sb[:, k * 2048:k * 2048 + rows],
                ).then_inc(s_hst[k % 2], 16)
            for s in range(NSLOT):
                scalar.wait_ge(s_smm, mm_after[cum[s + 1] - 1])
                if s >= 2:
                    # o_sb slot s%2 (written by relu s-2) is read by store (s-2)//2
                    k0 = (s - 2) // 2
                    scalar.wait_ge(s_ost[k0 % 2], 16 * (k0 // 2 + 1))
                scalar.activation(
                    o_sb[:, s % 2, :], po[s % 4][:, 0:128],
                    mybir.ActivationFunctionType.Relu,
                ).then_inc(s_act, 1)

    nc.compile()
    return nc


def _decode_out(oc):
    """[128, NSLOT*128] partition-major -> [NSLOT*128 rows, 128] fp32."""
    return np.ascontiguousarray(
        oc.reshape(128, NSLOT, 128).transpose(1, 0, 2).reshape(NSLOT * 128, 128)
    ).astype(np.float32)


def _run(x, edge_index, weight, bias, trace=False):
    common, per_core, slot_block, meta = _host_prep(x, edge_index, weight, bias)
    nc = _build_program(meta)
    in_maps = [dict(common, **per_core[c]) for c in range(NCORES)]
    res = run_bass_kernel_spmd(nc, in_maps, list(range(NCORES)), trace=trace)
    out = np.zeros((NBLK * 128, FOUT), np.float32)
    for c in range(NCORES):
        oc = _decode_out(np.asarray(res.results[c]["out"]))
        for s in range(NSLOT):
            b = slot_block[c, s]
            if b >= 0:
                out[b * 128:(b + 1) * 128] = oc[s * 128:(s + 1) * 128]
    return np.ascontiguousarray(out[:N_NODES]), res


def kernel(x, edge_index, weight, bias):
    out, _ = _run(x, edge_index, weight, bias, trace=False)
    return out


# revision 37
# speedup vs baseline: 2.0242x; 1.0696x over previous
"""GNN message-passing (graph convolution) kernel for 8 Trainium2 NeuronCores.

    out = relu(segment_sum(h[col], row) + bias),  h = x @ W

Strategy (dst-block sharding -- no collectives needed):
  * Host sorts edges by destination node and buckets them into 157 blocks of
    128 dst nodes.  Blocks are sorted by edge count and dealt snake-wise into
    20 slots x 8 cores so that slot s holds 8 similarly-sized blocks; the
    per-slot chunk count pb_s = max ceil(cnt/128) over its blocks is a program
    constant shared by all cores (SPMD), minimizing chunk padding.  Within a
    slot, edges are sorted by source node so early gathers depend only on a
    prefix of h.
  * Phase A (per core, replicated): h = x @ W on the PE in fp16 (PSUM fp32
    accumulate, 4 tiles per bank).  x is shipped pre-transposed [kk, tile, k,
    n] so the whole 10.3MB loads in 16 large DMAs through an 8-deep SBUF
    ring; DVE copies PSUM->SBUF; the ACT engine's HWDGE queue stores h to
    DRAM in 10 batched writes, each unlocking more of the gather stream.
  * Phase B: SWDGE dma_gather (HBM source) fetches 1024 edge rows per gather
    into val [128e, 8, 128f] slabs (4-deep ring); the DVE builds one-hot
    tiles S[e,n] = (iota == rowloc) (16-deep ring); the PE accumulates
    out_slot += S^T @ val over all chunks of the slot in PSUM fp32 -- an
    exact segment-sum.  The bias is folded in as one extra matmul per slot
    with constant operands (identity x bias-broadcast): no gather, no DVE.
    Gather idx tables are wrapped [16, n] and replicated only x2 (the SWDGE
    Q7 pair reads partitions 0-31; the rest is memset to 0 for the sim).
  * ACT applies ReLU PSUM->SBUF fp16; output stores are partition-major
    (512B runs, two slots per DMA).  The host scatters block rows back.

Numerics: fp16 operands with fp32 accumulation everywhere; the one-hot
matmul is exact, so the only error is fp16 rounding of x, W, h and the
output (~5e-4 relative).
"""

import sys

import numpy as np

sys.path.insert(0, "/opt/trn_rl_repo")

import concourse.bacc as bacc  # noqa: E402
import concourse.mybir as mybir  # noqa: E402
from concourse.bass_utils import run_bass_kernel_spmd  # noqa: E402

N_NODES = 20000
FIN = 256
FOUT = 128
N_EDGES = 640000

NTP = 158                # node tiles of 128 (padded even) -- h rows 20224
NPT = NTP // 2           # pair-tiles of 256 nodes: partition p holds 2p, 2p+1
NPAD = NTP * 128
NBLK = 157               # dst blocks of 128 nodes
NCORES = 8
NSLOT = 20               # block slots per core (slot 19: 5 real + 3 dummy)
NIDX = 1024              # idxs per dma_gather (8 chunks)
CPG = NIDX // 128        # chunks per gather
SCRATCH = 16384          # stock SWDGE ring (1024 descriptors)

FP16 = mybir.dt.float16
FP32 = mybir.dt.float32
I16 = mybir.dt.int16


def _plan(edge_index):
    """Sort/bucket edges; derive the SPMD-uniform slot structure."""
    row = np.asarray(edge_index[0]).astype(np.int64)
    col = np.asarray(edge_index[1]).astype(np.int64)
    order = np.argsort(row, kind="stable")
    rs = row[order].astype(np.int32)
    cs = col[order].astype(np.int32)

    blk = rs >> 7
    counts = np.bincount(blk, minlength=NBLK)
    starts = np.concatenate([[0], np.cumsum(counts)])

    big_first = np.argsort(counts, kind="stable")[::-1]  # block ids by size desc
    pbs = []
    slot_block = np.full((NCORES, NSLOT), -1, np.int64)
    for s in range(NSLOT):
        grp = big_first[s * NCORES:(s + 1) * NCORES]
        pbs.append(int(((counts[grp] + 127) // 128).max()))
        for c, b in enumerate(grp):
            slot_block[c, s] = b
    cum = np.concatenate([[0], np.cumsum(pbs)])
    nch = int(cum[-1])
    cpg = NIDX // 128
    ng = (nch + cpg - 1) // cpg
    nchp = ng * cpg
    return rs, cs, starts, slot_block, pbs, cum, nch, nchp, ng


def _host_prep(x, edge_index, weight, bias):
    """Cast/retile operands; build per-core gather index / rowloc tables."""
    x = np.asarray(x, np.float32)
    weight = np.asarray(weight, np.float32)
    bias = np.asarray(bias, np.float32)

    rs, cs, starts, slot_block, pbs, cum, nch, nchp, ng = _plan(edge_index)

    xpad = np.zeros((NPAD, FIN), np.float32)
    xpad[:N_NODES] = x
    # pair-tile layout: node pt*256 + 2p + e lives on partition p, so h rows
    # 2p, 2p+1 are adjacent in SBUF free dim -> 512B h-store descriptors.
    # xt[kk, pt*512 + k*256 + e*128 + p] = x[pt*256 + 2p + e, k*128 + kk]
    xt = np.ascontiguousarray(
        xpad.reshape(NPT, 128, 2, 2, 128)     # [pt, p, e, k, kk]
        .transpose(4, 0, 3, 2, 1)              # [kk, pt, k, e, p]
        .reshape(128, NPT * 512)
        .astype(np.float16)
    )
    # cst[:, 0:2, :] = W chunks; 2: iota; 3: identity; 4: bias broadcast
    cst = np.zeros((128, 5, 128), np.float16)
    cst[:, 0:2, :] = weight.reshape(2, 128, 128).transpose(1, 0, 2)
    cst[:, 2, :] = np.arange(128, dtype=np.float16)[None, :]
    cst[:, 3, :] = np.eye(128, dtype=np.float16)
    cst[:, 4, :] = bias.astype(np.float16)[None, :]

    gmax = np.zeros(ng, np.int64)
    col16 = np.zeros((NCORES, 2, 32, ng * (NIDX // 16)), np.int16)
    rl32 = np.full((NCORES, 128, nchp), -1.0, np.float32)
    for c in range(NCORES):
        lin_col = np.zeros(nchp * 128, np.int32)
        lin_rl = np.full(nchp * 128, -1.0, np.float32)
        for s in range(NSLOT):
            b = slot_block[c, s]
            if b < 0:
                continue
            e0, e1 = int(starts[b]), int(starts[b + 1])
            k = e1 - e0
            j0 = int(cum[s]) * 128
            o = np.argsort(cs[e0:e1], kind="stable")
            lin_col[j0:j0 + k] = cs[e0:e1][o]
            lin_rl[j0:j0 + k] = (rs[e0:e1] - b * 128)[o]
        # SWDGE idx layout: idx i -> partition i%16, column i//16 (x8 repl.)
        wrap2 = np.tile(
            lin_col.reshape(nchp * 128 // 16, 16).T.astype(np.int16), (2, 1)
        )
        col16[c][0] = wrap2
        col16[c][1] = wrap2
        rl32[c] = lin_rl.reshape(nchp, 128).T
        gmax = np.maximum(gmax, lin_col.reshape(ng, NIDX).max(axis=1))

    # per-gather h-frontier gate: h DRAM stores of 16 tiles (2048 rows)
    gates = [int(v) for v in (gmax // 2048 + 1)]
    meta = dict(
        pbs=pbs, cum=[int(v) for v in cum], nch=nch, nchp=nchp, ng=ng,
        gates=gates,
    )
    common = {"xt": xt, "cst": cst}
    per_core = [
        {"col": np.ascontiguousarray(col16[c]), "rl": np.ascontiguousarray(rl32[c])}
        for c in range(NCORES)
    ]
    return common, per_core, slot_block, meta


def _build_program(meta):
    pbs, cum = meta["pbs"], meta["cum"]
    nch, nchp, ng = meta["nch"], meta["nchp"], meta["ng"]
    gates = meta["gates"]
    chunk_slot = []                  # chunk j -> (slot, c)
    for s in range(NSLOT):
        for c in range(pbs[s]):
            chunk_slot.append((s, c))

    # cumulative segsum+bias matmul count after chunk j
    mm_after = []
    tot = 0
    for j in range(nch):
        s, c = chunk_slot[j]
        tot += 1
        if c == pbs[s] - 1:
            tot += 1
        mm_after.append(tot)

    NST = (NTP * 128 + 2047) // 2048  # h DRAM stores (16 tiles each)

    nc = bacc.Bacc("TRN2", dynamic_dma_scratch_size=SCRATCH, num_swdge_queues=2)

    xt_d = nc.dram_tensor("xt", [128, NPT * 512], FP16, kind="ExternalInput")
    cst_d = nc.dram_tensor("cst", [128, 5, 128], FP16, kind="ExternalInput")
    col_d = nc.dram_tensor("col", [2, 32, ng * (NIDX // 16)], I16, kind="ExternalInput")
    rl_d = nc.dram_tensor("rl", [128, nchp], FP32, kind="ExternalInput")
    h_d = nc.dram_tensor("hbuf", [NTP * 128, 128], FP16)
    o_d = nc.dram_tensor("out", [128, NSLOT * 128], FP16, kind="ExternalOutput")

    from contextlib import ExitStack

    with ExitStack() as es:
        pha = [es.enter_context(nc.psum_tensor(f"pha{k}", [128, 512], FP32)) for k in range(4)]
        po = [es.enter_context(nc.psum_tensor(f"po{k}", [128, 512], FP32)) for k in range(4)]
        xt_sb = es.enter_context(nc.sbuf_tensor("xt_sb", [128, 8, 5, 2, 2, 128], FP16))
        cst_sb = es.enter_context(nc.sbuf_tensor("cst_sb", [128, 5, 128], FP16))
        h_sb = es.enter_context(nc.sbuf_tensor("h_sb", [128, NTP * 128], FP16))
        val_eb = es.enter_context(nc.sbuf_tensor("val_eb", [128, 4, CPG, 128], FP16))
        s_sb = es.enter_context(nc.sbuf_tensor("s_sb", [128, 16, 128], FP16))
        o_sb = es.enter_context(nc.sbuf_tensor("o_sb", [128, 2, 128], FP16))
        col_sb = es.enter_context(nc.sbuf_tensor("col_sb", [128, ng * (NIDX // 16)], I16))
        rl_sb = es.enter_context(nc.sbuf_tensor("rl_sb", [128, nchp], FP32))

        s_x = [es.enter_context(nc.semaphore(f"s_x{k}")) for k in range(8)]
        s_ld = es.enter_context(nc.semaphore("s_ld"))
        s_msk = es.enter_context(nc.semaphore("s_msk"))
        s_hmm = es.enter_context(nc.semaphore("s_hmm"))
        s_hcp = es.enter_context(nc.semaphore("s_hcp"))
        s_hst = [es.enter_context(nc.semaphore(f"s_hst{k}")) for k in range(2)]
        s_gat = [es.enter_context(nc.semaphore(f"s_gat{k}")) for k in range(4)]
        s_s = es.enter_context(nc.semaphore("s_s"))
        s_smm = es.enter_context(nc.semaphore("s_smm"))
        s_act = es.enter_context(nc.semaphore("s_act"))
        s_ost = [es.enter_context(nc.semaphore(f"s_ost{k}")) for k in range(2)]
        block = es.enter_context(nc.Block())

        @block.sync
        def _(sync):
            sync.dma_start(cst_sb[:, :, :], cst_d[:, :, :]).then_inc(s_ld, 16)
            sync.dma_start(col_sb[0:32, :], col_d[:, :]).then_inc(s_ld, 16)
            sync.dma_start(rl_sb[:, :], rl_d[:, :]).then_inc(s_ld, 16)
            for L in range(16):
                if L >= 8:
                    sync.wait_ge(s_hmm, 10 * (L - 7))
                npt = min(5, NPT - 5 * L)
                sync.dma_start(
                    xt_sb[:, L % 8, 0:npt, :, :, :],
                    xt_d[:, L * 2560:L * 2560 + npt * 512],
                ).then_inc(s_x[L % 8], 16)
            for k in range(NSLOT // 2):
                sync.wait_ge(s_act, 2 * (k + 1))
                if k >= 2:
                    sync.wait_ge(s_ost[k % 2], 16 * (k // 2))
                sync.dma_start(
                    o_d[:, k * 256:(k + 1) * 256], o_sb[:, :, :]
                ).then_inc(s_ost[k % 2], 16)

        @block.gpsimd
        def _(gpsimd):
            gpsimd.wait_ge(s_ld, 48)
            gpsimd.wait_ge(s_msk, 2)
            for g in range(ng):
                st = gates[g]
                gpsimd.wait_ge(s_hst[0], 16 * ((st + 1) // 2))
                if st >= 2:
                    gpsimd.wait_ge(s_hst[1], 16 * (st // 2))
                if g >= 4:
                    gpsimd.wait_ge(s_smm, mm_after[CPG * (g - 3) - 1])
                # last gather: only its real chunks
                nix = min(NIDX, (nch - CPG * g) * 128)
                gpsimd.dma_gather(
                    val_eb[:, g % 4, 0:nix // 128, :],
                    h_d[0:min(gates[g] * 2048, NTP * 128), :],
                    col_sb[:, g * (NIDX // 16):g * (NIDX // 16) + nix // 16],
                    nix,
                    nix,
                    128,
                    queue_num=g % 2,
                ).then_inc(s_gat[g % 4], 16)

        @block.tensor
        def _(tensor):
            tensor.wait_ge(s_ld, 48)
            # phase A: two pair-tiles (512 nodes) per PSUM bank
            for pt in range(NPT):
                L = pt // 5
                if pt % 5 == 0:
                    tensor.wait_ge(s_x[L % 8], 16 * (L // 8 + 1))
                b = pt // 2
                if pt % 2 == 0 and b >= 4:
                    tensor.wait_ge(s_hcp, b - 3)
                for e in range(2):
                    col = (pt % 2) * 256 + e * 128
                    tensor.matmul(
                        pha[b % 4][:, col:col + 128],
                        xt_sb[:, L % 8, pt % 5, 0, e, :],
                        cst_sb[:, 0, :],
                        start=True, stop=False,
                    )
                    tensor.matmul(
                        pha[b % 4][:, col:col + 128],
                        xt_sb[:, L % 8, pt % 5, 1, e, :],
                        cst_sb[:, 1, :],
                        start=False, stop=True,
                    ).then_inc(s_hmm, 1)
            # phase B: segment-sum straight off each gathered slab
            for k in range(ng):
                tensor.wait_ge(s_gat[k % 4], 16 * (k // 4 + 1))
                for jj in range(CPG * k, CPG * k + CPG):
                    if jj >= nch:
                        break
                    s, c = chunk_slot[jj]
                    tensor.wait_ge(s_s, jj + 1)
                    if c == 0 and s >= 4:
                        tensor.wait_ge(s_act, s - 3)
                    tensor.matmul(
                        po[s % 4][:, 0:128],
                        s_sb[:, jj % 16, :],
                        val_eb[:, k % 4, jj % CPG, :],
                        start=(c == 0), stop=False,
                    ).then_inc(s_smm, 1)
                    if c == pbs[s] - 1:
                        tensor.matmul(
                            po[s % 4][:, 0:128],
                            cst_sb[:, 3, :],
                            cst_sb[:, 4, :],
                            start=False, stop=True,
                        ).then_inc(s_smm, 1)

        @block.vector
        def _(vector):
            # top idx partitions are never read by SWDGE; zero them so the
            # interp's bounds assert sees valid values
            vector.memset(col_sb[32:64, :], 0).then_inc(s_msk, 1)
            vector.memset(col_sb[96:128, :], 0).then_inc(s_msk, 1)
            vector.wait_ge(s_ld, 48)
            # phase A: PSUM fp32 -> SBUF fp16, 2 pair-tiles per copy
            for b in range((NPT + 1) // 2):
                npt = min(2, NPT - 2 * b)
                vector.wait_ge(s_hmm, 4 * b + 2 * npt)
                vector.tensor_copy(
                    h_sb[:, b * 512:b * 512 + npt * 256],
                    pha[b % 4][:, 0:npt * 256],
                ).then_inc(s_hcp, 1)
            # phase B: one-hot tiles S[e, n] = (iota[n] == rowloc[e])
            for j in range(nch):
                if j >= 16:
                    vector.wait_ge(s_smm, mm_after[j - 16])
                vector.tensor_scalar(
                    s_sb[:, j % 16, :],
                    cst_sb[:, 2, :],
                    rl_sb[:, j:j + 1],
                    None,
                    mybir.AluOpType.is_equal,
                ).then_inc(s_s, 1)

        @block.scalar
        def _(scalar):
            # h DRAM stores on the otherwise-idle ACT hwdge queue
            for k in range(NST):
                rows = min(2048, NTP * 128 - k * 2048)
                scalar.wait_ge(s_hcp, min(4 * (k + 1), (NPT + 1) // 2))
                if k >= 2:
                    scalar.wait_ge(s_hst[k % 2], 16 * (k // 2))
                scalar.dma_start(
                    h_d[k * 2048:k * 2048 + rows, :].rearrange(
                        "(t p e) f -> p t (e f)", p=128, e=2
                    ),
                    h_sb[:, k * 2048:k * 2048 + rows],
                ).then_inc(s_hst[k % 2], 16)
            for s in range(NSLOT):
                scalar.wait_ge(s_smm, mm_after[cum[s + 1] - 1])
                if s >= 2:
                    # o_sb slot s%2 (written by relu s-2) is read by store (s-2)//2
                    k0 = (s - 2) // 2
                    scalar.wait_ge(s_ost[k0 % 2], 16 * (k0 // 2 + 1))
                scalar.activation(
                    o_sb[:, s % 2, :], po[s % 4][:, 0:128],
                    mybir.ActivationFunctionType.Relu,
                ).then_inc(s_act, 1)

    nc.compile()
    return nc


def _decode_out(oc):
    """[128, NSLOT*128] partition-major -> [NSLOT*128 rows, 128] fp32."""
    return np.ascontiguousarray(
        oc.reshape(128, NSLOT, 128).transpose(1, 0, 2).reshape(NSLOT * 128, 128)
    ).astype(np.float32)


def _run(x, edge_index, weight, bias, trace=False):
    common, per_core, slot_block, meta = _host_prep(x, edge_index, weight, bias)
    nc = _build_program(meta)
    in_maps = [dict(common, **per_core[c]) for c in range(NCORES)]
    res = run_bass_kernel_spmd(nc, in_maps, list(range(NCORES)), trace=trace)
    out = np.zeros((NBLK * 128, FOUT), np.float32)
    for c in range(NCORES):
        oc = _decode_out(np.asarray(res.results[c]["out"]))
        for s in range(NSLOT):
            b = slot_block[c, s]
            if b >= 0:
                out[b * 128:(b + 1) * 128] = oc[s * 128:(s + 1) * 128]
    return np.ascontiguousarray(out[:N_NODES]), res


def kernel(x, edge_index, weight, bias):
    out, _ = _run(x, edge_index, weight, bias, trace=False)
    return out


# revision 41
# speedup vs baseline: 2.0349x; 1.0053x over previous
"""GNN message-passing (graph convolution) kernel for 8 Trainium2 NeuronCores.

    out = relu(segment_sum(h[col], row) + bias),  h = x @ W

Strategy (dst-block sharding -- no collectives needed):
  * Host sorts edges by destination node and buckets them into 157 blocks of
    128 dst nodes.  Blocks are sorted by edge count and dealt snake-wise into
    20 slots x 8 cores so that slot s holds 8 similarly-sized blocks; the
    per-slot chunk count pb_s = max ceil(cnt/128) over its blocks is a program
    constant shared by all cores (SPMD), minimizing chunk padding.  Within a
    slot, edges are sorted by source node so early gathers depend only on a
    prefix of h.
  * Phase A (per core, replicated): h = x @ W on the PE in fp16 (PSUM fp32
    accumulate, 4 tiles per bank).  x is shipped pre-transposed [kk, tile, k,
    n] so the whole 10.3MB loads in 16 large DMAs through an 8-deep SBUF
    ring; DVE copies PSUM->SBUF; the ACT engine's HWDGE queue stores h to
    DRAM in 10 batched writes, each unlocking more of the gather stream.
  * Phase B: SWDGE dma_gather (HBM source) fetches 1024 edge rows per gather
    into val [128e, 8, 128f] slabs (4-deep ring); the DVE builds one-hot
    tiles S[e,n] = (iota == rowloc) (16-deep ring); the PE accumulates
    out_slot += S^T @ val over all chunks of the slot in PSUM fp32 -- an
    exact segment-sum.  The bias is folded in as one extra matmul per slot
    with constant operands (identity x bias-broadcast): no gather, no DVE.
    Gather idx tables are wrapped [16, n] and replicated only x2 (the SWDGE
    Q7 pair reads partitions 0-31; the rest is memset to 0 for the sim).
  * ACT applies ReLU PSUM->SBUF fp16; output stores are partition-major
    (512B runs, two slots per DMA).  The host scatters block rows back.

Numerics: fp16 operands with fp32 accumulation everywhere; the one-hot
matmul is exact, so the only error is fp16 rounding of x, W, h and the
output (~5e-4 relative).
"""

import sys

import numpy as np

sys.path.insert(0, "/opt/trn_rl_repo")

import concourse.bacc as bacc  # noqa: E402
import concourse.mybir as mybir  # noqa: E402
from concourse.bass_utils import run_bass_kernel_spmd  # noqa: E402

N_NODES = 20000
FIN = 256
FOUT = 128
N_EDGES = 640000

NTP = 158                # node tiles of 128 (padded even) -- h rows 20224
NPT = NTP // 2           # pair-tiles of 256 nodes: partition p holds 2p, 2p+1
NPAD = NTP * 128
NBLK = 157               # dst blocks of 128 nodes
NCORES = 8
NSLOT = 20               # block slots per core (slot 19: 5 real + 3 dummy)
NIDX = 1024              # idxs per dma_gather (8 chunks)
CPG = NIDX // 128        # chunks per gather
SCRATCH = 16384          # stock SWDGE ring (1024 descriptors)

FP16 = mybir.dt.float16
FP32 = mybir.dt.float32
I16 = mybir.dt.int16


def _plan(edge_index):
    """Sort/bucket edges; derive the SPMD-uniform slot structure."""
    row = np.asarray(edge_index[0]).astype(np.int64)
    col = np.asarray(edge_index[1]).astype(np.int64)
    order = np.argsort(row, kind="stable")
    rs = row[order].astype(np.int32)
    cs = col[order].astype(np.int32)

    blk = rs >> 7
    counts = np.bincount(blk, minlength=NBLK)
    starts = np.concatenate([[0], np.cumsum(counts)])

    big_first = np.argsort(counts, kind="stable")[::-1]  # block ids by size desc
    pbs = []
    slot_block = np.full((NCORES, NSLOT), -1, np.int64)
    for s in range(NSLOT):
        grp = big_first[s * NCORES:(s + 1) * NCORES]
        pbs.append(int(((counts[grp] + 127) // 128).max()))
        for c, b in enumerate(grp):
            slot_block[c, s] = b
    cum = np.concatenate([[0], np.cumsum(pbs)])
    nch = int(cum[-1])
    cpg = NIDX // 128
    ng = (nch + cpg - 1) // cpg
    nchp = ng * cpg
    return rs, cs, starts, slot_block, pbs, cum, nch, nchp, ng


def _host_prep(x, edge_index, weight, bias):
    """Cast/retile operands; build per-core gather index / rowloc tables."""
    x = np.asarray(x, np.float32)
    weight = np.asarray(weight, np.float32)
    bias = np.asarray(bias, np.float32)

    rs, cs, starts, slot_block, pbs, cum, nch, nchp, ng = _plan(edge_index)

    xpad = np.zeros((NPAD, FIN), np.float32)
    xpad[:N_NODES] = x
    # pair-tile layout: node pt*256 + 2p + e lives on partition p, so h rows
    # 2p, 2p+1 are adjacent in SBUF free dim -> 512B h-store descriptors.
    # xt[kk, pt*512 + k*256 + e*128 + p] = x[pt*256 + 2p + e, k*128 + kk]
    xt = np.ascontiguousarray(
        xpad.reshape(NPT, 128, 2, 2, 128)     # [pt, p, e, k, kk]
        .transpose(4, 0, 3, 2, 1)              # [kk, pt, k, e, p]
        .reshape(128, NPT * 512)
        .astype(np.float16)
    )
    # cst[:, 0:2, :] = W chunks; 2: iota; 3: identity; 4: bias broadcast
    cst = np.zeros((128, 5, 128), np.float16)
    cst[:, 0:2, :] = weight.reshape(2, 128, 128).transpose(1, 0, 2)
    cst[:, 2, :] = np.arange(128, dtype=np.float16)[None, :]
    cst[:, 3, :] = np.eye(128, dtype=np.float16)
    cst[:, 4, :] = bias.astype(np.float16)[None, :]

    gmax = np.zeros(ng, np.int64)
    col16 = np.zeros((NCORES, 32, ng * (NIDX // 16)), np.int16)
    rl32 = np.full((NCORES, 128, nchp), -1.0, np.float32)
    for c in range(NCORES):
        lin_col = np.zeros(nchp * 128, np.int32)
        lin_rl = np.full(nchp * 128, -1.0, np.float32)
        for s in range(NSLOT):
            b = slot_block[c, s]
            if b < 0:
                continue
            e0, e1 = int(starts[b]), int(starts[b + 1])
            k = e1 - e0
            j0 = int(cum[s]) * 128
            o = np.argsort(cs[e0:e1], kind="stable")
            lin_col[j0:j0 + k] = cs[e0:e1][o]
            lin_rl[j0:j0 + k] = (rs[e0:e1] - b * 128)[o]
        # SWDGE idx layout: idx i -> partition i%16, column i//16 (x8 repl.)
        col16[c] = np.tile(
            lin_col.reshape(nchp * 128 // 16, 16).T.astype(np.int16), (2, 1)
        )
        rl32[c] = lin_rl.reshape(nchp, 128).T
        gmax = np.maximum(gmax, lin_col.reshape(ng, NIDX).max(axis=1))

    # per-gather h-frontier gate: h DRAM stores of 16 tiles (2048 rows)
    gates = [int(v) for v in (gmax // 2048 + 1)]
    meta = dict(
        pbs=pbs, cum=[int(v) for v in cum], nch=nch, nchp=nchp, ng=ng,
        gates=gates,
    )
    common = {"xt": xt, "cst": cst}
    per_core = [
        {"col": np.ascontiguousarray(col16[c]), "rl": np.ascontiguousarray(rl32[c])}
        for c in range(NCORES)
    ]
    return common, per_core, slot_block, meta


def _build_program(meta):
    pbs, cum = meta["pbs"], meta["cum"]
    nch, nchp, ng = meta["nch"], meta["nchp"], meta["ng"]
    gates = meta["gates"]
    chunk_slot = []                  # chunk j -> (slot, c)
    for s in range(NSLOT):
        for c in range(pbs[s]):
            chunk_slot.append((s, c))

    # cumulative segsum+bias matmul count after chunk j
    mm_after = []
    tot = 0
    for j in range(nch):
        s, c = chunk_slot[j]
        tot += 1
        if c == pbs[s] - 1:
            tot += 1
        mm_after.append(tot)

    NST = (NTP * 128 + 2047) // 2048  # h DRAM stores (16 tiles each)

    nc = bacc.Bacc("TRN2", dynamic_dma_scratch_size=SCRATCH)

    xt_d = nc.dram_tensor("xt", [128, NPT * 512], FP16, kind="ExternalInput")
    cst_d = nc.dram_tensor("cst", [128, 5, 128], FP16, kind="ExternalInput")
    col_d = nc.dram_tensor("col", [32, ng * (NIDX // 16)], I16, kind="ExternalInput")
    rl_d = nc.dram_tensor("rl", [128, nchp], FP32, kind="ExternalInput")
    h_d = nc.dram_tensor("hbuf", [NTP * 128, 128], FP16)
    o_d = nc.dram_tensor("out", [128, NSLOT * 128], FP16, kind="ExternalOutput")

    from contextlib import ExitStack

    with ExitStack() as es:
        pha = [es.enter_context(nc.psum_tensor(f"pha{k}", [128, 512], FP32)) for k in range(4)]
        po = [es.enter_context(nc.psum_tensor(f"po{k}", [128, 512], FP32)) for k in range(4)]
        xt_sb = es.enter_context(nc.sbuf_tensor("xt_sb", [128, 8, 5, 2, 2, 128], FP16))
        cst_sb = es.enter_context(nc.sbuf_tensor("cst_sb", [128, 5, 128], FP16))
        h_sb = es.enter_context(nc.sbuf_tensor("h_sb", [128, NTP * 128], FP16))
        val_eb = es.enter_context(nc.sbuf_tensor("val_eb", [128, 4, CPG, 128], FP16))
        s_sb = es.enter_context(nc.sbuf_tensor("s_sb", [128, 16, 128], FP16))
        o_sb = es.enter_context(nc.sbuf_tensor("o_sb", [128, 2, 128], FP16))
        col_sb = es.enter_context(nc.sbuf_tensor("col_sb", [128, ng * (NIDX // 16)], I16))
        rl_sb = es.enter_context(nc.sbuf_tensor("rl_sb", [128, nchp], FP32))

        s_x = [es.enter_context(nc.semaphore(f"s_x{k}")) for k in range(8)]
        s_ld = es.enter_context(nc.semaphore("s_ld"))
        s_msk = es.enter_context(nc.semaphore("s_msk"))
        s_hmm = es.enter_context(nc.semaphore("s_hmm"))
        s_hcp = es.enter_context(nc.semaphore("s_hcp"))
        s_hst = [es.enter_context(nc.semaphore(f"s_hst{k}")) for k in range(2)]
        s_gat = [es.enter_context(nc.semaphore(f"s_gat{k}")) for k in range(4)]
        s_s = es.enter_context(nc.semaphore("s_s"))
        s_smm = es.enter_context(nc.semaphore("s_smm"))
        s_act = es.enter_context(nc.semaphore("s_act"))
        s_ost = [es.enter_context(nc.semaphore(f"s_ost{k}")) for k in range(2)]
        block = es.enter_context(nc.Block())

        @block.sync
        def _(sync):
            sync.dma_start(cst_sb[:, :, :], cst_d[:, :, :]).then_inc(s_ld, 16)
            sync.dma_start(col_sb[0:32, :], col_d[:, :]).then_inc(s_ld, 16)
            sync.dma_start(rl_sb[:, :], rl_d[:, :]).then_inc(s_ld, 16)
            for L in range(16):
                if L >= 8:
                    sync.wait_ge(s_hmm, 10 * (L - 7))
                npt = min(5, NPT - 5 * L)
                sync.dma_start(
                    xt_sb[:, L % 8, 0:npt, :, :, :],
                    xt_d[:, L * 2560:L * 2560 + npt * 512],
                ).then_inc(s_x[L % 8], 16)
            for k in range(NSLOT // 2):
                sync.wait_ge(s_act, 2 * (k + 1))
                if k >= 2:
                    sync.wait_ge(s_ost[k % 2], 16 * (k // 2))
                sync.dma_start(
                    o_d[:, k * 256:(k + 1) * 256], o_sb[:, :, :]
                ).then_inc(s_ost[k % 2], 16)

        @block.gpsimd
        def _(gpsimd):
            gpsimd.wait_ge(s_ld, 48)
            gpsimd.wait_ge(s_msk, 2)
            for g in range(ng):
                st = gates[g]
                gpsimd.wait_ge(s_hst[0], 16 * ((st + 1) // 2))
                if st >= 2:
                    gpsimd.wait_ge(s_hst[1], 16 * (st // 2))
                if g >= 4:
                    gpsimd.wait_ge(s_smm, mm_after[CPG * (g - 3) - 1])
                # last gather: only its real chunks
                nix = min(NIDX, (nch - CPG * g) * 128)
                gpsimd.dma_gather(
                    val_eb[:, g % 4, 0:nix // 128, :],
                    h_d[0:min(gates[g] * 2048, NTP * 128), :],
                    col_sb[:, g * (NIDX // 16):g * (NIDX // 16) + nix // 16],
                    nix,
                    nix,
                    128,
                ).then_inc(s_gat[g % 4], 16)

        @block.tensor
        def _(tensor):
            tensor.wait_ge(s_ld, 48)
            # phase A: two pair-tiles (512 nodes) per PSUM bank
            for pt in range(NPT):
                L = pt // 5
                if pt % 5 == 0:
                    tensor.wait_ge(s_x[L % 8], 16 * (L // 8 + 1))
                b = pt // 2
                if pt % 2 == 0 and b >= 4:
                    tensor.wait_ge(s_hcp, b - 3)
                for e in range(2):
                    col = (pt % 2) * 256 + e * 128
                    tensor.matmul(
                        pha[b % 4][:, col:col + 128],
                        xt_sb[:, L % 8, pt % 5, 0, e, :],
                        cst_sb[:, 0, :],
                        start=True, stop=False,
                    )
                    tensor.matmul(
                        pha[b % 4][:, col:col + 128],
                        xt_sb[:, L % 8, pt % 5, 1, e, :],
                        cst_sb[:, 1, :],
                        start=False, stop=True,
                    ).then_inc(s_hmm, 1)
            # phase B: segment-sum straight off each gathered slab
            for k in range(ng):
                tensor.wait_ge(s_gat[k % 4], 16 * (k // 4 + 1))
                for jj in range(CPG * k, CPG * k + CPG):
                    if jj >= nch:
                        break
                    s, c = chunk_slot[jj]
                    tensor.wait_ge(s_s, jj + 1)
                    if c == 0 and s >= 4:
                        tensor.wait_ge(s_act, s - 3)
                    tensor.matmul(
                        po[s % 4][:, 0:128],
                        s_sb[:, jj % 16, :],
                        val_eb[:, k % 4, jj % CPG, :],
                        start=(c == 0), stop=False,
                    ).then_inc(s_smm, 1)
                    if c == pbs[s] - 1:
                        tensor.matmul(
                            po[s % 4][:, 0:128],
                            cst_sb[:, 3, :],
                            cst_sb[:, 4, :],
                            start=False, stop=True,
                        ).then_inc(s_smm, 1)

        @block.vector
        def _(vector):
            # top idx partitions are never read by SWDGE; zero them so the
            # interp's bounds assert sees valid values
            vector.memset(col_sb[32:64, :], 0).then_inc(s_msk, 1)
            vector.memset(col_sb[64:128, :], 0).then_inc(s_msk, 1)
            vector.wait_ge(s_ld, 48)
            # phase A: PSUM fp32 -> SBUF fp16, 2 pair-tiles per copy
            for b in range((NPT + 1) // 2):
                npt = min(2, NPT - 2 * b)
                vector.wait_ge(s_hmm, 4 * b + 2 * npt)
                vector.tensor_copy(
                    h_sb[:, b * 512:b * 512 + npt * 256],
                    pha[b % 4][:, 0:npt * 256],
                ).then_inc(s_hcp, 1)
            # phase B: one-hot tiles S[e, n] = (iota[n] == rowloc[e])
            for j in range(nch):
                if j >= 16:
                    vector.wait_ge(s_smm, mm_after[j - 16])
                vector.tensor_scalar(
                    s_sb[:, j % 16, :],
                    cst_sb[:, 2, :],
                    rl_sb[:, j:j + 1],
                    None,
                    mybir.AluOpType.is_equal,
                ).then_inc(s_s, 1)

        @block.scalar
        def _(scalar):
            # h DRAM stores on the otherwise-idle ACT hwdge queue
            for k in range(NST):
                rows = min(2048, NTP * 128 - k * 2048)
                scalar.wait_ge(s_hcp, min(4 * (k + 1), (NPT + 1) // 2))
                if k >= 2:
                    scalar.wait_ge(s_hst[k % 2], 16 * (k // 2))
                scalar.dma_start(
                    h_d[k * 2048:k * 2048 + rows, :].rearrange(
                        "(t p e) f -> p t (e f)", p=128, e=2
                    ),
                    h_sb[:, k * 2048:k * 2048 + rows],
                ).then_inc(s_hst[k % 2], 16)
            for s in range(NSLOT):
                scalar.wait_ge(s_smm, mm_after[cum[s + 1] - 1])
                if s >= 2:
                    # o_sb slot s%2 (written by relu s-2) is read by store (s-2)//2
                    k0 = (s - 2) // 2
                    scalar.wait_ge(s_ost[k0 % 2], 16 * (k0 // 2 + 1))
                scalar.activation(
                    o_sb[:, s % 2, :], po[s % 4][:, 0:128],
                    mybir.ActivationFunctionType.Relu,
                ).then_inc(s_act, 1)

    nc.compile()
    return nc


def _decode_out(oc):
    """[128, NSLOT*128] partition-major -> [NSLOT*128 rows, 128] fp32."""
    return np.ascontiguousarray(
        oc.reshape(128, NSLOT, 128).transpose(1, 0, 2).reshape(NSLOT * 128, 128)
    ).astype(np.float32)


def _run(x, edge_index, weight, bias, trace=False):
    common, per_core, slot_block, meta = _host_prep(x, edge_index, weight, bias)
    nc = _build_program(meta)
    in_maps = [dict(common, **per_core[c]) for c in range(NCORES)]
    res = run_bass_kernel_spmd(nc, in_maps, list(range(NCORES)), trace=trace)
    out = np.zeros((NBLK * 128, FOUT), np.float32)
    for c in range(NCORES):
        oc = _decode_out(np.asarray(res.results[c]["out"]))
        for s in range(NSLOT):
            b = slot_block[c, s]
            if b >= 0:
                out[b * 128:(b + 1) * 128] = oc[s * 128:(s + 1) * 128]
    return np.ascontiguousarray(out[:N_NODES]), res


def kernel(x, edge_index, weight, bias):
    out, _ = _run(x, edge_index, weight, bias, trace=False)
    return out


# revision 45
# speedup vs baseline: 2.0464x; 1.0057x over previous
"""GNN message-passing (graph convolution) kernel for 8 Trainium2 NeuronCores.

    out = relu(segment_sum(h[col], row) + bias),  h = x @ W

Strategy (dst-block sharding -- no collectives needed):
  * Host sorts edges by destination node and buckets them into 157 blocks of
    128 dst nodes.  Blocks are sorted by edge count and dealt snake-wise into
    20 slots x 8 cores so that slot s holds 8 similarly-sized blocks; the
    per-slot chunk count pb_s = max ceil(cnt/128) over its blocks is a program
    constant shared by all cores (SPMD), minimizing chunk padding.  Within a
    slot, edges are sorted by source node so early gathers depend only on a
    prefix of h.
  * Phase A (per core, replicated): h = x @ W on the PE in fp16 (PSUM fp32
    accumulate).  Nodes are pair-interleaved (partition p holds nodes 2p,
    2p+1 of each 256-node pair-tile) so h rows land adjacent in SBUF free
    dim and the DRAM h store runs with 512B descriptors (full DMA rate).
    x is shipped pre-transposed [kk, pt, k, e, p] and loads in 16 large DMAs
    through an 8-deep SBUF ring; PSUM->SBUF copies alternate DVE/ACT; the
    ACT engine's HWDGE queue stores h to DRAM in 10 batched writes, each
    unlocking more of the col-sorted gather stream.
  * Phase B: SWDGE dma_gather (HBM source) fetches 1024 edge rows per gather
    into val [128e, 8, 128f] slabs (4-deep ring); the DVE builds one-hot
    tiles S[e,n] = (iota == rowloc) (16-deep ring); the PE accumulates
    out_slot += S^T @ val over all chunks of the slot in PSUM fp32 -- an
    exact segment-sum.  The bias is folded in as one extra matmul per slot
    with constant operands (identity x bias-broadcast): no gather, no DVE.
    Gather idx tables are wrapped [16, n] and replicated only x2 (the SWDGE
    Q7 pair reads partitions 0-31; the rest is memset to 0 for the sim).
  * ACT applies ReLU PSUM->SBUF fp16; output stores are partition-major
    (512B runs, two slots per DMA).  The host scatters block rows back.

Numerics: fp16 operands with fp32 accumulation everywhere; the one-hot
matmul is exact, so the only error is fp16 rounding of x, W, h and the
output (~5e-4 relative).
"""

import sys

import numpy as np

sys.path.insert(0, "/opt/trn_rl_repo")

import concourse.bacc as bacc  # noqa: E402
import concourse.mybir as mybir  # noqa: E402
from concourse.bass_utils import run_bass_kernel_spmd  # noqa: E402

N_NODES = 20000
FIN = 256
FOUT = 128
N_EDGES = 640000

NTP = 158                # node tiles of 128 (padded even) -- h rows 20224
NPT = NTP // 2           # pair-tiles of 256 nodes: partition p holds 2p, 2p+1
NPAD = NTP * 128
NBLK = 157               # dst blocks of 128 nodes
NCORES = 8
NSLOT = 20               # block slots per core (slot 19: 5 real + 3 dummy)
NIDX = 1024              # idxs per dma_gather (8 chunks)
CPG = NIDX // 128        # chunks per gather
SCRATCH = 16384          # stock SWDGE ring (1024 descriptors)

FP16 = mybir.dt.float16
FP32 = mybir.dt.float32
I16 = mybir.dt.int16


def _plan(edge_index):
    """Sort/bucket edges; derive the SPMD-uniform slot structure."""
    row = np.asarray(edge_index[0]).astype(np.int64)
    col = np.asarray(edge_index[1]).astype(np.int64)
    order = np.argsort(row, kind="stable")
    rs = row[order].astype(np.int32)
    cs = col[order].astype(np.int32)

    blk = rs >> 7
    counts = np.bincount(blk, minlength=NBLK)
    starts = np.concatenate([[0], np.cumsum(counts)])

    big_first = np.argsort(counts, kind="stable")[::-1]  # block ids by size desc
    pbs = []
    slot_block = np.full((NCORES, NSLOT), -1, np.int64)
    for s in range(NSLOT):
        grp = big_first[s * NCORES:(s + 1) * NCORES]
        pbs.append(int(((counts[grp] + 127) // 128).max()))
        for c, b in enumerate(grp):
            slot_block[c, s] = b
    cum = np.concatenate([[0], np.cumsum(pbs)])
    nch = int(cum[-1])
    cpg = NIDX // 128
    ng = (nch + cpg - 1) // cpg
    nchp = ng * cpg
    return rs, cs, starts, slot_block, pbs, cum, nch, nchp, ng


def _host_prep(x, edge_index, weight, bias):
    """Cast/retile operands; build per-core gather index / rowloc tables."""
    x = np.asarray(x, np.float32)
    weight = np.asarray(weight, np.float32)
    bias = np.asarray(bias, np.float32)

    rs, cs, starts, slot_block, pbs, cum, nch, nchp, ng = _plan(edge_index)

    xpad = np.zeros((NPAD, FIN), np.float32)
    xpad[:N_NODES] = x
    # pair-tile layout: node pt*256 + 2p + e lives on partition p, so h rows
    # 2p, 2p+1 are adjacent in SBUF free dim -> 512B h-store descriptors.
    # xt[kk, pt*512 + k*256 + e*128 + p] = x[pt*256 + 2p + e, k*128 + kk]
    xt = np.ascontiguousarray(
        xpad.reshape(NPT, 128, 2, 2, 128)     # [pt, p, e, k, kk]
        .transpose(4, 0, 3, 2, 1)              # [kk, pt, k, e, p]
        .reshape(128, NPT * 512)
        .astype(np.float16)
    )
    # cst[:, 0:2, :] = W chunks; 2: iota; 3: identity; 4: bias broadcast
    cst = np.zeros((128, 5, 128), np.float16)
    cst[:, 0:2, :] = weight.reshape(2, 128, 128).transpose(1, 0, 2)
    cst[:, 2, :] = np.arange(128, dtype=np.float16)[None, :]
    cst[:, 3, :] = np.eye(128, dtype=np.float16)
    cst[:, 4, :] = bias.astype(np.float16)[None, :]

    gmax = np.zeros(ng, np.int64)
    col16 = np.zeros((NCORES, 32, ng * (NIDX // 16)), np.int16)
    rl32 = np.full((NCORES, 128, nchp), -1.0, np.float32)
    for c in range(NCORES):
        lin_col = np.zeros(nchp * 128, np.int32)
        lin_rl = np.full(nchp * 128, -1.0, np.float32)
        for s in range(NSLOT):
            b = slot_block[c, s]
            if b < 0:
                continue
            e0, e1 = int(starts[b]), int(starts[b + 1])
            k = e1 - e0
            j0 = int(cum[s]) * 128
            o = np.argsort(cs[e0:e1], kind="stable")
            lin_col[j0:j0 + k] = cs[e0:e1][o]
            lin_rl[j0:j0 + k] = (rs[e0:e1] - b * 128)[o]
        # SWDGE idx layout: idx i -> partition i%16, column i//16 (x8 repl.)
        col16[c] = np.tile(
            lin_col.reshape(nchp * 128 // 16, 16).T.astype(np.int16), (2, 1)
        )
        rl32[c] = lin_rl.reshape(nchp, 128).T
        gmax = np.maximum(gmax, lin_col.reshape(ng, NIDX).max(axis=1))

    # per-gather h-frontier gate: h DRAM stores of 16 tiles (2048 rows)
    gates = [int(v) for v in (gmax // 2048 + 1)]
    meta = dict(
        pbs=pbs, cum=[int(v) for v in cum], nch=nch, nchp=nchp, ng=ng,
        gates=gates,
    )
    common = {"xt": xt, "cst": cst}
    per_core = [
        {"col": np.ascontiguousarray(col16[c]), "rl": np.ascontiguousarray(rl32[c])}
        for c in range(NCORES)
    ]
    return common, per_core, slot_block, meta


def _build_program(meta):
    pbs, cum = meta["pbs"], meta["cum"]
    nch, nchp, ng = meta["nch"], meta["nchp"], meta["ng"]
    gates = meta["gates"]
    chunk_slot = []                  # chunk j -> (slot, c)
    for s in range(NSLOT):
        for c in range(pbs[s]):
            chunk_slot.append((s, c))

    # cumulative segsum+bias matmul count after chunk j
    mm_after = []
    tot = 0
    for j in range(nch):
        s, c = chunk_slot[j]
        tot += 1
        if c == pbs[s] - 1:
            tot += 1
        mm_after.append(tot)

    NST = (NTP * 128 + 2047) // 2048  # h DRAM stores (16 tiles each)

    nc = bacc.Bacc("TRN2", dynamic_dma_scratch_size=SCRATCH)

    xt_d = nc.dram_tensor("xt", [128, NPT * 512], FP16, kind="ExternalInput")
    cst_d = nc.dram_tensor("cst", [128, 5, 128], FP16, kind="ExternalInput")
    col_d = nc.dram_tensor("col", [32, ng * (NIDX // 16)], I16, kind="ExternalInput")
    rl_d = nc.dram_tensor("rl", [128, nchp], FP32, kind="ExternalInput")
    h_d = nc.dram_tensor("hbuf", [NTP * 128, 128], FP16)
    o_d = nc.dram_tensor("out", [128, NSLOT * 128], FP16, kind="ExternalOutput")

    from contextlib import ExitStack

    with ExitStack() as es:
        pha = [es.enter_context(nc.psum_tensor(f"pha{k}", [128, 512], FP32)) for k in range(4)]
        po = [es.enter_context(nc.psum_tensor(f"po{k}", [128, 512], FP32)) for k in range(4)]
        xt_sb = es.enter_context(nc.sbuf_tensor("xt_sb", [128, 8, 5, 2, 2, 128], FP16))
        cst_sb = es.enter_context(nc.sbuf_tensor("cst_sb", [128, 5, 128], FP16))
        h_sb = es.enter_context(nc.sbuf_tensor("h_sb", [128, NTP * 128], FP16))
        val_eb = es.enter_context(nc.sbuf_tensor("val_eb", [128, 8, CPG, 128], FP16))
        s_sb = es.enter_context(nc.sbuf_tensor("s_sb", [128, 16, 128], FP16))
        o_sb = es.enter_context(nc.sbuf_tensor("o_sb", [128, 2, 128], FP16))
        col_sb = es.enter_context(nc.sbuf_tensor("col_sb", [128, ng * (NIDX // 16)], I16))
        rl_sb = es.enter_context(nc.sbuf_tensor("rl_sb", [128, nchp], FP32))

        s_x = [es.enter_context(nc.semaphore(f"s_x{k}")) for k in range(8)]
        s_ld = es.enter_context(nc.semaphore("s_ld"))
        s_msk = es.enter_context(nc.semaphore("s_msk"))
        s_hmm = es.enter_context(nc.semaphore("s_hmm"))
        s_hcp = es.enter_context(nc.semaphore("s_hcp"))
        s_hst = [es.enter_context(nc.semaphore(f"s_hst{k}")) for k in range(4)]
        s_gat = [es.enter_context(nc.semaphore(f"s_gat{k}")) for k in range(8)]
        s_s = es.enter_context(nc.semaphore("s_s"))
        s_smm = es.enter_context(nc.semaphore("s_smm"))
        s_act = es.enter_context(nc.semaphore("s_act"))
        s_ost = [es.enter_context(nc.semaphore(f"s_ost{k}")) for k in range(2)]
        block = es.enter_context(nc.Block())

        @block.sync
        def _(sync):
            sync.dma_start(cst_sb[:, :, :], cst_d[:, :, :]).then_inc(s_ld, 16)
            sync.dma_start(col_sb[0:32, :], col_d[:, :]).then_inc(s_ld, 16)
            sync.dma_start(rl_sb[:, :], rl_d[:, :]).then_inc(s_ld, 16)
            for L in range(16):
                if L >= 8:
                    sync.wait_ge(s_hmm, 10 * (L - 7))
                npt = min(5, NPT - 5 * L)
                sync.dma_start(
                    xt_sb[:, L % 8, 0:npt, :, :, :],
                    xt_d[:, L * 2560:L * 2560 + npt * 512],
                ).then_inc(s_x[L % 8], 16)
            for k in range(NSLOT // 2):
                sync.wait_ge(s_act, 2 * (k + 1))
                if k >= 2:
                    sync.wait_ge(s_ost[k % 2], 16 * (k // 2))
                sync.dma_start(
                    o_d[:, k * 256:(k + 1) * 256], o_sb[:, :, :]
                ).then_inc(s_ost[k % 2], 16)

        @block.gpsimd
        def _(gpsimd):
            gpsimd.wait_ge(s_ld, 48)
            gpsimd.wait_ge(s_msk, 2)
            for g in range(ng):
                st = gates[g]
                for p in range(4):
                    cnt = len([k for k in range(st) if k % 4 == p])
                    if cnt:
                        gpsimd.wait_ge(s_hst[p], 16 * cnt)
                if g >= 8:
                    gpsimd.wait_ge(s_smm, mm_after[CPG * (g - 7) - 1])
                # last gather: only its real chunks
                nix = min(NIDX, (nch - CPG * g) * 128)
                gpsimd.dma_gather(
                    val_eb[:, g % 8, 0:nix // 128, :],
                    h_d[0:min(gates[g] * 2048, NTP * 128), :],
                    col_sb[:, g * (NIDX // 16):g * (NIDX // 16) + nix // 16],
                    nix,
                    nix,
                    128,
                ).then_inc(s_gat[g % 8], 16)

        @block.tensor
        def _(tensor):
            tensor.wait_ge(s_ld, 48)
            # phase A: two pair-tiles (512 nodes) per PSUM bank
            for pt in range(NPT):
                L = pt // 5
                if pt % 5 == 0:
                    tensor.wait_ge(s_x[L % 8], 16 * (L // 8 + 1))
                b = pt // 2
                if pt % 2 == 0 and b >= 4:
                    tensor.wait_ge(s_hcp, b - 3)
                for e in range(2):
                    col = (pt % 2) * 256 + e * 128
                    tensor.matmul(
                        pha[b % 4][:, col:col + 128],
                        xt_sb[:, L % 8, pt % 5, 0, e, :],
                        cst_sb[:, 0, :],
                        start=True, stop=False,
                    )
                    tensor.matmul(
                        pha[b % 4][:, col:col + 128],
                        xt_sb[:, L % 8, pt % 5, 1, e, :],
                        cst_sb[:, 1, :],
                        start=False, stop=True,
                    ).then_inc(s_hmm, 1)
            # phase B: segment-sum straight off each gathered slab
            for k in range(ng):
                tensor.wait_ge(s_gat[k % 8], 16 * (k // 8 + 1))
                for jj in range(CPG * k, CPG * k + CPG):
                    if jj >= nch:
                        break
                    s, c = chunk_slot[jj]
                    tensor.wait_ge(s_s, jj + 1)
                    if c == 0 and s >= 4:
                        tensor.wait_ge(s_act, s - 3)
                    tensor.matmul(
                        po[s % 4][:, 0:128],
                        s_sb[:, jj % 16, :],
                        val_eb[:, k % 8, jj % CPG, :],
                        start=(c == 0), stop=False,
                    ).then_inc(s_smm, 1)
                    if c == pbs[s] - 1:
                        tensor.matmul(
                            po[s % 4][:, 0:128],
                            cst_sb[:, 3, :],
                            cst_sb[:, 4, :],
                            start=False, stop=True,
                        ).then_inc(s_smm, 1)

        @block.vector
        def _(vector):
            # top idx partitions are never read by SWDGE; zero them so the
            # interp's bounds assert sees valid values
            vector.memset(col_sb[32:64, :], 0).then_inc(s_msk, 1)
            vector.memset(col_sb[64:128, :], 0).then_inc(s_msk, 1)
            vector.wait_ge(s_ld, 48)
            # phase A: PSUM fp32 -> SBUF fp16, 2 pair-tiles per copy
            for b in range((NPT + 1) // 2):
                npt = min(2, NPT - 2 * b)
                vector.wait_ge(s_hmm, 4 * b + 2 * npt)
                vector.tensor_copy(
                    h_sb[:, b * 512:b * 512 + npt * 256],
                    pha[b % 4][:, 0:npt * 256],
                ).then_inc(s_hcp, 1)
            # phase B: one-hot tiles S[e, n] = (iota[n] == rowloc[e])
            for j in range(nch):
                if j >= 16:
                    vector.wait_ge(s_smm, mm_after[j - 16])
                vector.tensor_scalar(
                    s_sb[:, j % 16, :],
                    cst_sb[:, 2, :],
                    rl_sb[:, j:j + 1],
                    None,
                    mybir.AluOpType.is_equal,
                ).then_inc(s_s, 1)

        @block.scalar
        def _(scalar):
            # h DRAM stores on the otherwise-idle ACT hwdge queue
            for k in range(NST):
                rows = min(2048, NTP * 128 - k * 2048)
                scalar.wait_ge(s_hcp, min(4 * (k + 1), (NPT + 1) // 2))
                if k >= 2:
                    scalar.wait_ge(s_hst[k % 2], 16 * (k // 2))
                scalar.dma_start(
                    h_d[k * 2048:k * 2048 + rows, :].rearrange(
                        "(t p e) f -> p t (e f)", p=128, e=2
                    ),
                    h_sb[:, k * 2048:k * 2048 + rows],
                ).then_inc(s_hst[k % 2], 16)
            for s in range(NSLOT):
                scalar.wait_ge(s_smm, mm_after[cum[s + 1] - 1])
                if s >= 2:
                    # o_sb slot s%2 (written by relu s-2) is read by store (s-2)//2
                    k0 = (s - 2) // 2
                    scalar.wait_ge(s_ost[k0 % 2], 16 * (k0 // 2 + 1))
                scalar.activation(
                    o_sb[:, s % 2, :], po[s % 4][:, 0:128],
                    mybir.ActivationFunctionType.Relu,
                ).then_inc(s_act, 1)

    nc.compile()
    return nc


def _decode_out(oc):
    """[128, NSLOT*128] partition-major -> [NSLOT*128 rows, 128] fp32."""
    return np.ascontiguousarray(
        oc.reshape(128, NSLOT, 128).transpose(1, 0, 2).reshape(NSLOT * 128, 128)
    ).astype(np.float32)


def _run(x, edge_index, weight, bias, trace=False):
    common, per_core, slot_block, meta = _host_prep(x, edge_index, weight, bias)
    nc = _build_program(meta)
    in_maps = [dict(common, **per_core[c]) for c in range(NCORES)]
    res = run_bass_kernel_spmd(nc, in_maps, list(range(NCORES)), trace=trace)
    out = np.zeros((NBLK * 128, FOUT), np.float32)
    for c in range(NCORES):
        oc = _decode_out(np.asarray(res.results[c]["out"]))
        for s in range(NSLOT):
            b = slot_block[c, s]
            if b >= 0:
                out[b * 128:(b + 1) * 128] = oc[s * 128:(s + 1) * 128]
    return np.ascontiguousarray(out[:N_NODES]), res


def kernel(x, edge_index, weight, bias):
    out, _ = _run(x, edge_index, weight, bias, trace=False)
    return out


# revision 46
# speedup vs baseline: 2.0612x; 1.0072x over previous
"""GNN message-passing (graph convolution) kernel for 8 Trainium2 NeuronCores.

    out = relu(segment_sum(h[col], row) + bias),  h = x @ W

Strategy (dst-block sharding -- no collectives needed):
  * Host sorts edges by destination node and buckets them into 157 blocks of
    128 dst nodes.  Blocks are sorted by edge count and dealt snake-wise into
    20 slots x 8 cores so that slot s holds 8 similarly-sized blocks; the
    per-slot chunk count pb_s = max ceil(cnt/128) over its blocks is a program
    constant shared by all cores (SPMD), minimizing chunk padding.  Within a
    slot, edges are sorted by source node so early gathers depend only on a
    prefix of h.
  * Phase A (per core, replicated): h = x @ W on the PE in fp16 (PSUM fp32
    accumulate).  Nodes are pair-interleaved (partition p holds nodes 2p,
    2p+1 of each 256-node pair-tile) so h rows land adjacent in SBUF free
    dim and the DRAM h store runs with 512B descriptors (full DMA rate).
    x is shipped pre-transposed [kk, pt, k, e, p] and loads in 16 large DMAs
    through an 8-deep SBUF ring; PSUM->SBUF copies alternate DVE/ACT; the
    ACT engine's HWDGE queue stores h to DRAM in 10 batched writes, each
    unlocking more of the col-sorted gather stream.
  * Phase B: SWDGE dma_gather (HBM source) fetches 1024 edge rows per gather
    into val [128e, 8, 128f] slabs (4-deep ring); the DVE builds one-hot
    tiles S[e,n] = (iota == rowloc) (16-deep ring); the PE accumulates
    out_slot += S^T @ val over all chunks of the slot in PSUM fp32 -- an
    exact segment-sum.  The bias is folded in as one extra matmul per slot
    with constant operands (identity x bias-broadcast): no gather, no DVE.
    Gather idx tables are wrapped [16, n] and replicated only x2 (the SWDGE
    Q7 pair reads partitions 0-31; the rest is memset to 0 for the sim).
  * ACT applies ReLU PSUM->SBUF fp16; output stores are partition-major
    (512B runs, two slots per DMA).  The host scatters block rows back.

Numerics: fp16 operands with fp32 accumulation everywhere; the one-hot
matmul is exact, so the only error is fp16 rounding of x, W, h and the
output (~5e-4 relative).
"""

import sys

import numpy as np

sys.path.insert(0, "/opt/trn_rl_repo")

import concourse.bacc as bacc  # noqa: E402
import concourse.mybir as mybir  # noqa: E402
from concourse.bass_utils import run_bass_kernel_spmd  # noqa: E402

N_NODES = 20000
FIN = 256
FOUT = 128
N_EDGES = 640000

NTP = 158                # node tiles of 128 (padded even) -- h rows 20224
NPT = NTP // 2           # pair-tiles of 256 nodes: partition p holds 2p, 2p+1
NPAD = NTP * 128
NBLK = 157               # dst blocks of 128 nodes
NCORES = 8
NSLOT = 20               # block slots per core (slot 19: 5 real + 3 dummy)
NIDX = 1024              # idxs per dma_gather (8 chunks)
CPG = NIDX // 128        # chunks per gather
SCRATCH = 16384          # stock SWDGE ring (1024 descriptors)

FP16 = mybir.dt.float16
FP32 = mybir.dt.float32
I16 = mybir.dt.int16


def _plan(edge_index):
    """Sort/bucket edges; derive the SPMD-uniform slot structure."""
    row = np.asarray(edge_index[0]).astype(np.int64)
    col = np.asarray(edge_index[1]).astype(np.int64)
    order = np.argsort(row, kind="stable")
    rs = row[order].astype(np.int32)
    cs = col[order].astype(np.int32)

    blk = rs >> 7
    counts = np.bincount(blk, minlength=NBLK)
    starts = np.concatenate([[0], np.cumsum(counts)])

    big_first = np.argsort(counts, kind="stable")[::-1]  # block ids by size desc
    pbs = []
    slot_block = np.full((NCORES, NSLOT), -1, np.int64)
    for s in range(NSLOT):
        grp = big_first[s * NCORES:(s + 1) * NCORES]
        pbs.append(int(((counts[grp] + 127) // 128).max()))
        for c, b in enumerate(grp):
            slot_block[c, s] = b
    cum = np.concatenate([[0], np.cumsum(pbs)])
    nch = int(cum[-1])
    cpg = NIDX // 128
    ng = (nch + cpg - 1) // cpg
    nchp = ng * cpg
    return rs, cs, starts, slot_block, pbs, cum, nch, nchp, ng


def _host_prep(x, edge_index, weight, bias):
    """Cast/retile operands; build per-core gather index / rowloc tables."""
    x = np.asarray(x, np.float32)
    weight = np.asarray(weight, np.float32)
    bias = np.asarray(bias, np.float32)

    rs, cs, starts, slot_block, pbs, cum, nch, nchp, ng = _plan(edge_index)

    xpad = np.zeros((NPAD, FIN), np.float32)
    xpad[:N_NODES] = x
    # pair-tile layout: node pt*256 + 2p + e lives on partition p, so h rows
    # 2p, 2p+1 are adjacent in SBUF free dim -> 512B h-store descriptors.
    # xt[kk, pt*512 + k*256 + e*128 + p] = x[pt*256 + 2p + e, k*128 + kk]
    xt = np.ascontiguousarray(
        xpad.reshape(NPT, 128, 2, 2, 128)     # [pt, p, e, k, kk]
        .transpose(4, 0, 3, 2, 1)              # [kk, pt, k, e, p]
        .reshape(128, NPT * 512)
        .astype(np.float16)
    )
    # cst[:, 0:2, :] = W chunks; 2: iota; 3: identity; 4: bias broadcast
    cst = np.zeros((128, 5, 128), np.float16)
    cst[:, 0:2, :] = weight.reshape(2, 128, 128).transpose(1, 0, 2)
    cst[:, 2, :] = np.arange(128, dtype=np.float16)[None, :]
    cst[:, 3, :] = np.eye(128, dtype=np.float16)
    cst[:, 4, :] = bias.astype(np.float16)[None, :]

    gmax = np.zeros(ng, np.int64)
    col16 = np.zeros((NCORES, 32, ng * (NIDX // 16)), np.int16)
    rl32 = np.full((NCORES, 128, nchp), -1.0, np.float32)
    for c in range(NCORES):
        lin_col = np.zeros(nchp * 128, np.int32)
        lin_rl = np.full(nchp * 128, -1.0, np.float32)
        for s in range(NSLOT):
            b = slot_block[c, s]
            if b < 0:
                continue
            e0, e1 = int(starts[b]), int(starts[b + 1])
            k = e1 - e0
            j0 = int(cum[s]) * 128
            o = np.argsort(cs[e0:e1], kind="stable")
            lin_col[j0:j0 + k] = cs[e0:e1][o]
            lin_rl[j0:j0 + k] = (rs[e0:e1] - b * 128)[o]
        # SWDGE idx layout: idx i -> partition i%16, column i//16 (x8 repl.)
        col16[c] = np.tile(
            lin_col.reshape(nchp * 128 // 16, 16).T.astype(np.int16), (2, 1)
        )
        rl32[c] = lin_rl.reshape(nchp, 128).T
        gmax = np.maximum(gmax, lin_col.reshape(ng, NIDX).max(axis=1))

    # per-gather h-frontier gate: h DRAM stores of 16 tiles (2048 rows)
    gates = [int(v) for v in (gmax // 2048 + 1)]
    meta = dict(
        pbs=pbs, cum=[int(v) for v in cum], nch=nch, nchp=nchp, ng=ng,
        gates=gates,
    )
    common = {"xt": xt, "cst": cst}
    per_core = [
        {"col": np.ascontiguousarray(col16[c]), "rl": np.ascontiguousarray(rl32[c])}
        for c in range(NCORES)
    ]
    return common, per_core, slot_block, meta


def _build_program(meta):
    pbs, cum = meta["pbs"], meta["cum"]
    nch, nchp, ng = meta["nch"], meta["nchp"], meta["ng"]
    gates = meta["gates"]
    chunk_slot = []                  # chunk j -> (slot, c)
    for s in range(NSLOT):
        for c in range(pbs[s]):
            chunk_slot.append((s, c))

    # cumulative segsum+bias matmul count after chunk j
    mm_after = []
    tot = 0
    for j in range(nch):
        s, c = chunk_slot[j]
        tot += 1
        if c == pbs[s] - 1:
            tot += 1
        mm_after.append(tot)

    NST = (NTP * 128 + 2047) // 2048  # h DRAM stores (16 tiles each)

    nc = bacc.Bacc("TRN2", dynamic_dma_scratch_size=SCRATCH)

    xt_d = nc.dram_tensor("xt", [128, NPT * 512], FP16, kind="ExternalInput")
    cst_d = nc.dram_tensor("cst", [128, 5, 128], FP16, kind="ExternalInput")
    col_d = nc.dram_tensor("col", [32, ng * (NIDX // 16)], I16, kind="ExternalInput")
    rl_d = nc.dram_tensor("rl", [128, nchp], FP32, kind="ExternalInput")
    h_d = nc.dram_tensor("hbuf", [NTP * 128, 128], FP16)
    o_d = nc.dram_tensor("out", [128, NSLOT * 128], FP16, kind="ExternalOutput")

    from contextlib import ExitStack

    with ExitStack() as es:
        pha = [es.enter_context(nc.psum_tensor(f"pha{k}", [128, 512], FP32)) for k in range(4)]
        po = [es.enter_context(nc.psum_tensor(f"po{k}", [128, 512], FP32)) for k in range(4)]
        xt_sb = es.enter_context(nc.sbuf_tensor("xt_sb", [128, 8, 5, 2, 2, 128], FP16))
        cst_sb = es.enter_context(nc.sbuf_tensor("cst_sb", [128, 5, 128], FP16))
        h_sb = es.enter_context(nc.sbuf_tensor("h_sb", [128, NTP * 128], FP16))
        val_eb = es.enter_context(nc.sbuf_tensor("val_eb", [128, 8, CPG, 128], FP16))
        s_sb = es.enter_context(nc.sbuf_tensor("s_sb", [128, 16, 128], FP16))
        o_sb = es.enter_context(nc.sbuf_tensor("o_sb", [128, 2, 128], FP16))
        col_sb = es.enter_context(nc.sbuf_tensor("col_sb", [128, ng * (NIDX // 16)], I16))
        rl_sb = es.enter_context(nc.sbuf_tensor("rl_sb", [128, nchp], FP32))

        s_x = [es.enter_context(nc.semaphore(f"s_x{k}")) for k in range(8)]
        s_ld = es.enter_context(nc.semaphore("s_ld"))
        s_msk = es.enter_context(nc.semaphore("s_msk"))
        s_hmm = es.enter_context(nc.semaphore("s_hmm"))
        s_hcp = es.enter_context(nc.semaphore("s_hcp"))
        s_hst = [es.enter_context(nc.semaphore(f"s_hst{k}")) for k in range(4)]
        s_gat = [es.enter_context(nc.semaphore(f"s_gat{k}")) for k in range(8)]
        s_s = es.enter_context(nc.semaphore("s_s"))
        s_prep = es.enter_context(nc.semaphore("s_prep"))
        s_smm = es.enter_context(nc.semaphore("s_smm"))
        s_act = es.enter_context(nc.semaphore("s_act"))
        s_ost = [es.enter_context(nc.semaphore(f"s_ost{k}")) for k in range(2)]
        block = es.enter_context(nc.Block())

        @block.sync
        def _(sync):
            sync.dma_start(cst_sb[:, :, :], cst_d[:, :, :]).then_inc(s_ld, 16)
            sync.dma_start(col_sb[0:32, :], col_d[:, :]).then_inc(s_ld, 16)
            sync.dma_start(rl_sb[:, :], rl_d[:, :]).then_inc(s_ld, 16)
            for L in range(16):
                if L >= 8:
                    sync.wait_ge(s_hmm, 10 * (L - 7))
                npt = min(5, NPT - 5 * L)
                sync.dma_start(
                    xt_sb[:, L % 8, 0:npt, :, :, :],
                    xt_d[:, L * 2560:L * 2560 + npt * 512],
                ).then_inc(s_x[L % 8], 16)
            for k in range(NSLOT // 2):
                sync.wait_ge(s_act, 2 * (k + 1))
                if k >= 2:
                    sync.wait_ge(s_ost[k % 2], 16 * (k // 2))
                sync.dma_start(
                    o_d[:, k * 256:(k + 1) * 256], o_sb[:, :, :]
                ).then_inc(s_ost[k % 2], 16)

        @block.gpsimd
        def _(gpsimd):
            gpsimd.wait_ge(s_ld, 48)
            gpsimd.wait_ge(s_msk, 2)
            g_star = next(
                (g for g in range(ng) if gates[g] >= NST), ng
            )
            for g in range(ng):
                st = gates[g]
                prep = g == g_star
                if prep:
                    # generate descriptors BEFORE the final h-store gate so
                    # the SWDGE gen overlaps the preceding transfer
                    nix = min(NIDX, (nch - CPG * g) * 128)
                    gpsimd.dma_gather(
                        val_eb[:, g % 8, 0:nix // 128, :],
                        h_d[0:min(gates[g] * 2048, NTP * 128), :],
                        col_sb[:, g * (NIDX // 16):g * (NIDX // 16) + nix // 16],
                        nix,
                        nix,
                        128,
                        prepare_only=True,
                        sem=s_gat[g % 8],
                    ).then_inc(s_prep, 1)
                    gpsimd.wait_ge(s_prep, 1)
                for p in range(4):
                    cnt = len([k for k in range(st) if k % 4 == p])
                    if cnt:
                        gpsimd.wait_ge(s_hst[p], 16 * cnt)
                if g >= 8:
                    gpsimd.wait_ge(s_smm, mm_after[CPG * (g - 7) - 1])
                if prep:
                    gpsimd.trigger_dma(count=1)
                    continue
                # last gather: only its real chunks
                nix = min(NIDX, (nch - CPG * g) * 128)
                gpsimd.dma_gather(
                    val_eb[:, g % 8, 0:nix // 128, :],
                    h_d[0:min(gates[g] * 2048, NTP * 128), :],
                    col_sb[:, g * (NIDX // 16):g * (NIDX // 16) + nix // 16],
                    nix,
                    nix,
                    128,
                ).then_inc(s_gat[g % 8], 16)

        @block.tensor
        def _(tensor):
            tensor.wait_ge(s_ld, 48)
            # phase A: two pair-tiles (512 nodes) per PSUM bank
            for pt in range(NPT):
                L = pt // 5
                if pt % 5 == 0:
                    tensor.wait_ge(s_x[L % 8], 16 * (L // 8 + 1))
                b = pt // 2
                if pt % 2 == 0 and b >= 4:
                    tensor.wait_ge(s_hcp, b - 3)
                for e in range(2):
                    col = (pt % 2) * 256 + e * 128
                    tensor.matmul(
                        pha[b % 4][:, col:col + 128],
                        xt_sb[:, L % 8, pt % 5, 0, e, :],
                        cst_sb[:, 0, :],
                        start=True, stop=False,
                    )
                    tensor.matmul(
                        pha[b % 4][:, col:col + 128],
                        xt_sb[:, L % 8, pt % 5, 1, e, :],
                        cst_sb[:, 1, :],
                        start=False, stop=True,
                    ).then_inc(s_hmm, 1)
            # phase B: segment-sum straight off each gathered slab
            for k in range(ng):
                tensor.wait_ge(s_gat[k % 8], 16 * (k // 8 + 1))
                for jj in range(CPG * k, CPG * k + CPG):
                    if jj >= nch:
                        break
                    s, c = chunk_slot[jj]
                    tensor.wait_ge(s_s, jj + 1)
                    if c == 0 and s >= 4:
                        tensor.wait_ge(s_act, s - 3)
                    tensor.matmul(
                        po[s % 4][:, 0:128],
                        s_sb[:, jj % 16, :],
                        val_eb[:, k % 8, jj % CPG, :],
                        start=(c == 0), stop=False,
                    ).then_inc(s_smm, 1)
                    if c == pbs[s] - 1:
                        tensor.matmul(
                            po[s % 4][:, 0:128],
                            cst_sb[:, 3, :],
                            cst_sb[:, 4, :],
                            start=False, stop=True,
                        ).then_inc(s_smm, 1)

        @block.vector
        def _(vector):
            # top idx partitions are never read by SWDGE; zero them so the
            # interp's bounds assert sees valid values
            vector.memset(col_sb[32:64, :], 0).then_inc(s_msk, 1)
            vector.memset(col_sb[64:128, :], 0).then_inc(s_msk, 1)
            vector.wait_ge(s_ld, 48)
            # phase A: PSUM fp32 -> SBUF fp16, 2 pair-tiles per copy
            for b in range((NPT + 1) // 2):
                npt = min(2, NPT - 2 * b)
                vector.wait_ge(s_hmm, 4 * b + 2 * npt)
                vector.tensor_copy(
                    h_sb[:, b * 512:b * 512 + npt * 256],
                    pha[b % 4][:, 0:npt * 256],
                ).then_inc(s_hcp, 1)
            # phase B: one-hot tiles S[e, n] = (iota[n] == rowloc[e])
            for j in range(nch):
                if j >= 16:
                    vector.wait_ge(s_smm, mm_after[j - 16])
                vector.tensor_scalar(
                    s_sb[:, j % 16, :],
                    cst_sb[:, 2, :],
                    rl_sb[:, j:j + 1],
                    None,
                    mybir.AluOpType.is_equal,
                ).then_inc(s_s, 1)

        @block.scalar
        def _(scalar):
            # h DRAM stores on the otherwise-idle ACT hwdge queue
            for k in range(NST):
                rows = min(2048, NTP * 128 - k * 2048)
                scalar.wait_ge(s_hcp, min(4 * (k + 1), (NPT + 1) // 2))
                if k >= 2:
                    scalar.wait_ge(s_hst[k % 2], 16 * (k // 2))
                scalar.dma_start(
                    h_d[k * 2048:k * 2048 + rows, :].rearrange(
                        "(t p e) f -> p t (e f)", p=128, e=2
                    ),
                    h_sb[:, k * 2048:k * 2048 + rows],
                ).then_inc(s_hst[k % 2], 16)
            for s in range(NSLOT):
                scalar.wait_ge(s_smm, mm_after[cum[s + 1] - 1])
                if s >= 2:
                    # o_sb slot s%2 (written by relu s-2) is read by store (s-2)//2
                    k0 = (s - 2) // 2
                    scalar.wait_ge(s_ost[k0 % 2], 16 * (k0 // 2 + 1))
                scalar.activation(
                    o_sb[:, s % 2, :], po[s % 4][:, 0:128],
                    mybir.ActivationFunctionType.Relu,
                ).then_inc(s_act, 1)

    nc.compile()
    return nc


def _decode_out(oc):
    """[128, NSLOT*128] partition-major -> [NSLOT*128 rows, 128] fp32."""
    return np.ascontiguousarray(
        oc.reshape(128, NSLOT, 128).transpose(1, 0, 2).reshape(NSLOT * 128, 128)
    ).astype(np.float32)


def _run(x, edge_index, weight, bias, trace=False):
    common, per_core, slot_block, meta = _host_prep(x, edge_index, weight, bias)
    nc = _build_program(meta)
    in_maps = [dict(common, **per_core[c]) for c in range(NCORES)]
    res = run_bass_kernel_spmd(nc, in_maps, list(range(NCORES)), trace=trace)
    out = np.zeros((NBLK * 128, FOUT), np.float32)
    for c in range(NCORES):
        oc = _decode_out(np.asarray(res.results[c]["out"]))
        for s in range(NSLOT):
            b = slot_block[c, s]
            if b >= 0:
                out[b * 128:(b + 1) * 128] = oc[s * 128:(s + 1) * 128]
    return np.ascontiguousarray(out[:N_NODES]), res


def kernel(x, edge_index, weight, bias):
    out, _ = _run(x, edge_index, weight, bias, trace=False)
    return out


# revision 48
# speedup vs baseline: 2.0680x; 1.0033x over previous
"""GNN message-passing (graph convolution) kernel for 8 Trainium2 NeuronCores.

    out = relu(segment_sum(h[col], row) + bias),  h = x @ W

Strategy (dst-block sharding -- no collectives needed):
  * Host sorts edges by destination node and buckets them into 157 blocks of
    128 dst nodes.  Blocks are sorted by edge count and dealt snake-wise into
    20 slots x 8 cores so that slot s holds 8 similarly-sized blocks; the
    per-slot chunk count pb_s = max ceil(cnt/128) over its blocks is a program
    constant shared by all cores (SPMD), minimizing chunk padding.  Within a
    slot, edges are sorted by source node so early gathers depend only on a
    prefix of h.
  * Phase A (per core, replicated): h = x @ W on the PE in fp16 (PSUM fp32
    accumulate).  Nodes are pair-interleaved (partition p holds nodes 2p,
    2p+1 of each 256-node pair-tile) so h rows land adjacent in SBUF free
    dim and the DRAM h store runs with 512B descriptors (full DMA rate).
    x is shipped pre-transposed [kk, pt, k, e, p] and loads in 16 large DMAs
    through an 8-deep SBUF ring; PSUM->SBUF copies alternate DVE/ACT; the
    ACT engine's HWDGE queue stores h to DRAM in 10 batched writes, each
    unlocking more of the col-sorted gather stream.
  * Phase B: SWDGE dma_gather (HBM source) fetches 1024 edge rows per gather
    into val [128e, 8, 128f] slabs (4-deep ring); the DVE builds one-hot
    tiles S[e,n] = (iota == rowloc) (16-deep ring); the PE accumulates
    out_slot += S^T @ val over all chunks of the slot in PSUM fp32 -- an
    exact segment-sum.  The bias is folded in as one extra matmul per slot
    with constant operands (identity x bias-broadcast): no gather, no DVE.
    Gather idx tables are wrapped [16, n] and replicated only x2 (the SWDGE
    Q7 pair reads partitions 0-31; the rest is memset to 0 for the sim).
  * ACT applies ReLU PSUM->SBUF fp16; output stores are partition-major
    (512B runs, two slots per DMA).  The host scatters block rows back.

Numerics: fp16 operands with fp32 accumulation everywhere; the one-hot
matmul is exact, so the only error is fp16 rounding of x, W, h and the
output (~5e-4 relative).
"""

import sys

import numpy as np

sys.path.insert(0, "/opt/trn_rl_repo")

import concourse.bacc as bacc  # noqa: E402
import concourse.mybir as mybir  # noqa: E402
from concourse.bass_utils import run_bass_kernel_spmd  # noqa: E402

N_NODES = 20000
FIN = 256
FOUT = 128
N_EDGES = 640000

NTP = 158                # node tiles of 128 (padded even) -- h rows 20224
NPT = NTP // 2           # pair-tiles of 256 nodes: partition p holds 2p, 2p+1
NPAD = NTP * 128
NBLK = 157               # dst blocks of 128 nodes
NCORES = 8
NSLOT = 20               # block slots per core (slot 19: 5 real + 3 dummy)
NIDX = 1024              # idxs per dma_gather (8 chunks)
CPG = NIDX // 128        # chunks per gather
SCRATCH = 16384          # stock SWDGE ring (1024 descriptors)

FP16 = mybir.dt.float16
FP32 = mybir.dt.float32
I16 = mybir.dt.int16
I8 = mybir.dt.int8


def _plan(edge_index):
    """Sort/bucket edges; derive the SPMD-uniform slot structure."""
    row = np.asarray(edge_index[0]).astype(np.int64)
    col = np.asarray(edge_index[1]).astype(np.int64)
    order = np.argsort(row, kind="stable")
    rs = row[order].astype(np.int32)
    cs = col[order].astype(np.int32)

    blk = rs >> 7
    counts = np.bincount(blk, minlength=NBLK)
    starts = np.concatenate([[0], np.cumsum(counts)])

    big_first = np.argsort(counts, kind="stable")[::-1]  # block ids by size desc
    pbs = []
    slot_block = np.full((NCORES, NSLOT), -1, np.int64)
    for s in range(NSLOT):
        grp = big_first[s * NCORES:(s + 1) * NCORES]
        pbs.append(int(((counts[grp] + 127) // 128).max()))
        for c, b in enumerate(grp):
            slot_block[c, s] = b
    cum = np.concatenate([[0], np.cumsum(pbs)])
    nch = int(cum[-1])
    cpg = NIDX // 128
    ng = (nch + cpg - 1) // cpg
    nchp = ng * cpg
    return rs, cs, starts, slot_block, pbs, cum, nch, nchp, ng


def _host_prep(x, edge_index, weight, bias):
    """Cast/retile operands; build per-core gather index / rowloc tables."""
    x = np.asarray(x, np.float32)
    weight = np.asarray(weight, np.float32)
    bias = np.asarray(bias, np.float32)

    rs, cs, starts, slot_block, pbs, cum, nch, nchp, ng = _plan(edge_index)

    xpad = np.zeros((NPAD, FIN), np.float32)
    xpad[:N_NODES] = x
    # pair-tile layout: node pt*256 + 2p + e lives on partition p, so h rows
    # 2p, 2p+1 are adjacent in SBUF free dim -> 512B h-store descriptors.
    # xt[kk, pt*512 + k*256 + e*128 + p] = x[pt*256 + 2p + e, k*128 + kk]
    xt = np.ascontiguousarray(
        xpad.reshape(NPT, 128, 2, 2, 128)     # [pt, p, e, k, kk]
        .transpose(4, 0, 3, 2, 1)              # [kk, pt, k, e, p]
        .reshape(128, NPT * 512)
        .astype(np.float16)
    )
    # cst[:, 0:2, :] = W chunks; 2: iota; 3: identity; 4: bias broadcast
    cst = np.zeros((128, 5, 128), np.float16)
    cst[:, 0:2, :] = weight.reshape(2, 128, 128).transpose(1, 0, 2)
    cst[:, 2, :] = np.arange(128, dtype=np.float16)[None, :]
    cst[:, 3, :] = np.eye(128, dtype=np.float16)
    cst[:, 4, :] = bias.astype(np.float16)[None, :]

    gmax = np.zeros(ng, np.int64)
    col16 = np.zeros((NCORES, 32, ng * (NIDX // 16)), np.int16)
    rl8 = np.full((NCORES, 128, nchp), -1, np.int8)
    for c in range(NCORES):
        lin_col = np.zeros(nchp * 128, np.int32)
        lin_rl = np.full(nchp * 128, -1.0, np.float32)
        for s in range(NSLOT):
            b = slot_block[c, s]
            if b < 0:
                continue
            e0, e1 = int(starts[b]), int(starts[b + 1])
            k = e1 - e0
            j0 = int(cum[s]) * 128
            o = np.argsort(cs[e0:e1], kind="stable")
            lin_col[j0:j0 + k] = cs[e0:e1][o]
            lin_rl[j0:j0 + k] = (rs[e0:e1] - b * 128)[o]
        # SWDGE idx layout: idx i -> partition i%16, column i//16 (x8 repl.)
        col16[c] = np.tile(
            lin_col.reshape(nchp * 128 // 16, 16).T.astype(np.int16), (2, 1)
        )
        rl8[c] = lin_rl.reshape(nchp, 128).T.astype(np.int8)
        gmax = np.maximum(gmax, lin_col.reshape(ng, NIDX).max(axis=1))

    # per-gather h-frontier gate: h DRAM stores of 16 tiles (2048 rows)
    gates = [int(v) for v in (gmax // 2048 + 1)]
    meta = dict(
        pbs=pbs, cum=[int(v) for v in cum], nch=nch, nchp=nchp, ng=ng,
        gates=gates,
    )
    common = {"xt": xt, "cst": cst}
    per_core = [
        {"col": np.ascontiguousarray(col16[c]), "rl": np.ascontiguousarray(rl8[c])}
        for c in range(NCORES)
    ]
    return common, per_core, slot_block, meta


def _build_program(meta):
    pbs, cum = meta["pbs"], meta["cum"]
    nch, nchp, ng = meta["nch"], meta["nchp"], meta["ng"]
    gates = meta["gates"]
    chunk_slot = []                  # chunk j -> (slot, c)
    for s in range(NSLOT):
        for c in range(pbs[s]):
            chunk_slot.append((s, c))

    # cumulative segsum+bias matmul count after chunk j
    mm_after = []
    tot = 0
    for j in range(nch):
        s, c = chunk_slot[j]
        tot += 1
        if c == pbs[s] - 1:
            tot += 1
        mm_after.append(tot)

    NST = (NTP * 128 + 2047) // 2048  # h DRAM stores (16 tiles each)

    nc = bacc.Bacc("TRN2", dynamic_dma_scratch_size=SCRATCH)

    xt_d = nc.dram_tensor("xt", [128, NPT * 512], FP16, kind="ExternalInput")
    cst_d = nc.dram_tensor("cst", [128, 5, 128], FP16, kind="ExternalInput")
    col_d = nc.dram_tensor("col", [32, ng * (NIDX // 16)], I16, kind="ExternalInput")
    rl_d = nc.dram_tensor("rl", [128, nchp], I8, kind="ExternalInput")
    h_d = nc.dram_tensor("hbuf", [NTP * 128, 128], FP16)
    o_d = nc.dram_tensor("out", [128, NSLOT * 128], FP16, kind="ExternalOutput")

    from contextlib import ExitStack

    with ExitStack() as es:
        pha = [es.enter_context(nc.psum_tensor(f"pha{k}", [128, 512], FP32)) for k in range(4)]
        po = [es.enter_context(nc.psum_tensor(f"po{k}", [128, 512], FP32)) for k in range(4)]
        xt_sb = es.enter_context(nc.sbuf_tensor("xt_sb", [128, 8, 5, 2, 2, 128], FP16))
        cst_sb = es.enter_context(nc.sbuf_tensor("cst_sb", [128, 5, 128], FP16))
        h_sb = es.enter_context(nc.sbuf_tensor("h_sb", [128, NTP * 128], FP16))
        val_eb = es.enter_context(nc.sbuf_tensor("val_eb", [128, 8, CPG, 128], FP16))
        s_sb = es.enter_context(nc.sbuf_tensor("s_sb", [128, 16, 128], FP16))
        o_sb = es.enter_context(nc.sbuf_tensor("o_sb", [128, 2, 128], FP16))
        col_sb = es.enter_context(nc.sbuf_tensor("col_sb", [128, ng * (NIDX // 16)], I16))
        rl8_sb = es.enter_context(nc.sbuf_tensor("rl8_sb", [128, nchp], I8))
        rl_sb = es.enter_context(nc.sbuf_tensor("rl_sb", [128, nchp], FP32))

        s_x = [es.enter_context(nc.semaphore(f"s_x{k}")) for k in range(8)]
        s_ld = es.enter_context(nc.semaphore("s_ld"))
        s_msk = es.enter_context(nc.semaphore("s_msk"))
        s_hmm = es.enter_context(nc.semaphore("s_hmm"))
        s_hcp = es.enter_context(nc.semaphore("s_hcp"))
        s_hst = [es.enter_context(nc.semaphore(f"s_hst{k}")) for k in range(4)]
        s_gat = [es.enter_context(nc.semaphore(f"s_gat{k}")) for k in range(8)]
        s_s = es.enter_context(nc.semaphore("s_s"))
        s_prep = es.enter_context(nc.semaphore("s_prep"))
        s_cvt = es.enter_context(nc.semaphore("s_cvt"))
        s_smm = es.enter_context(nc.semaphore("s_smm"))
        s_act = es.enter_context(nc.semaphore("s_act"))
        s_ost = [es.enter_context(nc.semaphore(f"s_ost{k}")) for k in range(2)]
        block = es.enter_context(nc.Block())

        @block.sync
        def _(sync):
            sync.dma_start(cst_sb[:, :, :], cst_d[:, :, :]).then_inc(s_ld, 16)
            sync.dma_start(col_sb[0:32, :], col_d[:, :]).then_inc(s_ld, 16)
            sync.dma_start(rl8_sb[:, :], rl_d[:, :]).then_inc(s_ld, 16)
            for L in range(16):
                if L >= 8:
                    sync.wait_ge(s_hmm, 10 * (L - 7))
                npt = min(5, NPT - 5 * L)
                sync.dma_start(
                    xt_sb[:, L % 8, 0:npt, :, :, :],
                    xt_d[:, L * 2560:L * 2560 + npt * 512],
                ).then_inc(s_x[L % 8], 16)
            for k in range(NSLOT // 2):
                sync.wait_ge(s_act, 2 * (k + 1))
                if k >= 2:
                    sync.wait_ge(s_ost[k % 2], 16 * (k // 2))
                sync.dma_start(
                    o_d[:, k * 256:(k + 1) * 256], o_sb[:, :, :]
                ).then_inc(s_ost[k % 2], 16)

        @block.gpsimd
        def _(gpsimd):
            gpsimd.wait_ge(s_ld, 48)
            gpsimd.wait_ge(s_msk, 2)
            g_star = next(
                (g for g in range(ng) if gates[g] >= NST), ng
            )
            for g in range(ng):
                st = gates[g]
                prep = g == g_star
                if prep:
                    # generate descriptors BEFORE the final h-store gate so
                    # the SWDGE gen overlaps the preceding transfer
                    nix = min(NIDX, (nch - CPG * g) * 128)
                    gpsimd.dma_gather(
                        val_eb[:, g % 8, 0:nix // 128, :],
                        h_d[0:min(gates[g] * 2048, NTP * 128), :],
                        col_sb[:, g * (NIDX // 16):g * (NIDX // 16) + nix // 16],
                        nix,
                        nix,
                        128,
                        prepare_only=True,
                        sem=s_gat[g % 8],
                    ).then_inc(s_prep, 1)
                    gpsimd.wait_ge(s_prep, 1)
                for p in range(4):
                    cnt = len([k for k in range(st) if k % 4 == p])
                    if cnt:
                        gpsimd.wait_ge(s_hst[p], 16 * cnt)
                if g >= 8:
                    gpsimd.wait_ge(s_smm, mm_after[CPG * (g - 7) - 1])
                if prep:
                    gpsimd.trigger_dma(count=1)
                    continue
                # last gather: only its real chunks
                nix = min(NIDX, (nch - CPG * g) * 128)
                gpsimd.dma_gather(
                    val_eb[:, g % 8, 0:nix // 128, :],
                    h_d[0:min(gates[g] * 2048, NTP * 128), :],
                    col_sb[:, g * (NIDX // 16):g * (NIDX // 16) + nix // 16],
                    nix,
                    nix,
                    128,
                ).then_inc(s_gat[g % 8], 16)

        @block.tensor
        def _(tensor):
            tensor.wait_ge(s_ld, 48)
            # phase A: two pair-tiles (512 nodes) per PSUM bank
            for pt in range(NPT):
                L = pt // 5
                if pt % 5 == 0:
                    tensor.wait_ge(s_x[L % 8], 16 * (L // 8 + 1))
                b = pt // 2
                if pt % 2 == 0 and b >= 4:
                    tensor.wait_ge(s_hcp, b - 3)
                for e in range(2):
                    col = (pt % 2) * 256 + e * 128
                    tensor.matmul(
                        pha[b % 4][:, col:col + 128],
                        xt_sb[:, L % 8, pt % 5, 0, e, :],
                        cst_sb[:, 0, :],
                        start=True, stop=False,
                    )
                    tensor.matmul(
                        pha[b % 4][:, col:col + 128],
                        xt_sb[:, L % 8, pt % 5, 1, e, :],
                        cst_sb[:, 1, :],
                        start=False, stop=True,
                    ).then_inc(s_hmm, 1)
            # phase B: segment-sum straight off each gathered slab
            for k in range(ng):
                tensor.wait_ge(s_gat[k % 8], 16 * (k // 8 + 1))
                for jj in range(CPG * k, CPG * k + CPG):
                    if jj >= nch:
                        break
                    s, c = chunk_slot[jj]
                    tensor.wait_ge(s_s, jj + 1)
                    if c == 0 and s >= 4:
                        tensor.wait_ge(s_act, s - 3)
                    tensor.matmul(
                        po[s % 4][:, 0:128],
                        s_sb[:, jj % 16, :],
                        val_eb[:, k % 8, jj % CPG, :],
                        start=(c == 0), stop=False,
                    ).then_inc(s_smm, 1)
                    if c == pbs[s] - 1:
                        tensor.matmul(
                            po[s % 4][:, 0:128],
                            cst_sb[:, 3, :],
                            cst_sb[:, 4, :],
                            start=False, stop=True,
                        ).then_inc(s_smm, 1)

        @block.vector
        def _(vector):
            # top idx partitions are never read by SWDGE; zero them so the
            # interp's bounds assert sees valid values
            vector.memset(col_sb[32:64, :], 0).then_inc(s_msk, 1)
            vector.memset(col_sb[64:128, :], 0).then_inc(s_msk, 1)
            vector.wait_ge(s_ld, 48)
            # phase A: PSUM fp32 -> SBUF fp16, 2 pair-tiles per copy
            for b in range((NPT + 1) // 2):
                npt = min(2, NPT - 2 * b)
                vector.wait_ge(s_hmm, 4 * b + 2 * npt)
                vector.tensor_copy(
                    h_sb[:, b * 512:b * 512 + npt * 256],
                    pha[b % 4][:, 0:npt * 256],
                ).then_inc(s_hcp, 1)
            # phase B: widen rowloc int8 -> fp32, then one-hot tiles
            vector.tensor_copy(rl_sb[:, :], rl8_sb[:, :]).then_inc(s_cvt, 1)
            vector.wait_ge(s_cvt, 1)
            for j in range(nch):
                if j >= 16:
                    vector.wait_ge(s_smm, mm_after[j - 16])
                vector.tensor_scalar(
                    s_sb[:, j % 16, :],
                    cst_sb[:, 2, :],
                    rl_sb[:, j:j + 1],
                    None,
                    mybir.AluOpType.is_equal,
                ).then_inc(s_s, 1)

        @block.scalar
        def _(scalar):
            # h DRAM stores on the otherwise-idle ACT hwdge queue
            for k in range(NST):
                rows = min(2048, NTP * 128 - k * 2048)
                scalar.wait_ge(s_hcp, min(4 * (k + 1), (NPT + 1) // 2))
                if k >= 2:
                    scalar.wait_ge(s_hst[k % 2], 16 * (k // 2))
                scalar.dma_start(
                    h_d[k * 2048:k * 2048 + rows, :].rearrange(
                        "(t p e) f -> p t (e f)", p=128, e=2
                    ),
                    h_sb[:, k * 2048:k * 2048 + rows],
                ).then_inc(s_hst[k % 2], 16)
            for s in range(NSLOT):
                scalar.wait_ge(s_smm, mm_after[cum[s + 1] - 1])
                if s >= 2:
                    # o_sb slot s%2 (written by relu s-2) is read by store (s-2)//2
                    k0 = (s - 2) // 2
                    scalar.wait_ge(s_ost[k0 % 2], 16 * (k0 // 2 + 1))
                scalar.activation(
                    o_sb[:, s % 2, :], po[s % 4][:, 0:128],
                    mybir.ActivationFunctionType.Relu,
                ).then_inc(s_act, 1)

    nc.compile()
    return nc


def _decode_out(oc):
    """[128, NSLOT*128] partition-major -> [NSLOT*128 rows, 128] fp32."""
    return np.ascontiguousarray(
        oc.reshape(128, NSLOT, 128).transpose(1, 0, 2).reshape(NSLOT * 128, 128)
    ).astype(np.float32)


def _run(x, edge_index, weight, bias, trace=False):
    common, per_core, slot_block, meta = _host_prep(x, edge_index, weight, bias)
    nc = _build_program(meta)
    in_maps = [dict(common, **per_core[c]) for c in range(NCORES)]
    res = run_bass_kernel_spmd(nc, in_maps, list(range(NCORES)), trace=trace)
    out = np.zeros((NBLK * 128, FOUT), np.float32)
    for c in range(NCORES):
        oc = _decode_out(np.asarray(res.results[c]["out"]))
        for s in range(NSLOT):
            b = slot_block[c, s]
            if b >= 0:
                out[b * 128:(b + 1) * 128] = oc[s * 128:(s + 1) * 128]
    return np.ascontiguousarray(out[:N_NODES]), res


def kernel(x, edge_index, weight, bias):
    out, _ = _run(x, edge_index, weight, bias, trace=False)
    return out
